# revision 26
# baseline (speedup 1.0000x reference)
"""MiniMax-M2 MoE kernel for 8 Trainium2 NeuronCores.

Single-launch expert-parallel design:
  Host (data movement / dispatch only): fp32 routing decides WHICH tokens go
    to WHICH expert (indices only); a planner cuts each expert's token list
    into at most two pieces and packs them into up to 4 static expert slots
    per core (capacities chosen to minimize modeled PE time, ~7% over the
    perfect-balance floor); tokens are gathered per slot and weights
    pre-transposed/cast to bf16.
  Device (all output-value arithmetic, one SPMD launch):
    - per slot, recompute router scores for the slot's gathered tokens
      (logits -> sigmoid -> top-4 threshold on bias-corrected scores ->
      renormalized combine weight of the slot's own expert; the host permutes
      the gate matrix per core so slot s's expert is always column s),
    - SwiGLU FFN (bf16 matmuls) and combine-weight scaling.  silu(g) is
      computed as g * sigmoid(g) so the Activation engine only ever needs the
      sigmoid table (one LoadActFuncSet instead of thrashing Silu<->Sigmoid).
    - stage 2 runs one chunk behind stage 1 (software pipeline) so the PE
      never idles on the ht evacuation chain.
  Host: scatter-add per-slot outputs into [T, H].
"""

import math

import ml_dtypes
import numpy as np

import concourse.bass as bass
import concourse.tile as tile
from concourse import bacc, mybir
from concourse.bass_utils import run_bass_kernel_spmd

T, H, F, E, TOPK = 4096, 1024, 512, 16, 4
NCORES = 8
KC = H // 128   # contraction chunks (hidden dim)
FC = F // 128   # stage-2 contraction chunks
F32 = mybir.dt.float32
BF16 = mybir.dt.bfloat16

_nc_cache: dict = {}
LAST_CAPS = (832, 492, 512, 354)  # caps used by the most recent kernel() call


def _plan_slots(counts: np.ndarray):
    """Choose per-core slot capacities and expert-piece placement.

    Experts are cut into at most two pieces (primary, remainder).  Slot type
    0 holds heavy-expert primaries (cap A), type 2 light primaries (cap C);
    the remainders are ranked and split between types 1 and 3.  The (A, C)
    cut points are searched to minimize modeled PE time: stage-1 cost scales
    with total capacity, stage-2/routing with ceil(cap/128) tiles.

    Returns (caps, placement) where placement[core] is a list of
    (expert, tok_start, length) per slot (length may be 0).
    """
    E_ = len(counts)
    order = np.argsort(-counts, kind="stable")
    heavy = [int(e) for e in order[:NCORES]]
    light = [int(e) for e in order[NCORES:]]
    c0 = int(counts[heavy[0]])
    c8 = int(counts[light[0]])

    def plan_cost(caps):
        ct = sum(caps)
        tiles = sum(math.ceil(cp / 128) for cp in caps if cp)
        return 64 * ct + (8 * 512 + 8 * E_) * tiles

    def build(A, C):
        pieces_b = []  # (expert, start, len) remainders
        for e in heavy:
            if counts[e] > A:
                pieces_b.append((e, A, int(counts[e]) - A))
        for e in light:
            if counts[e] > C:
                pieces_b.append((e, C, int(counts[e]) - C))
        if len(pieces_b) > 2 * NCORES:
            return None
        pieces_b.sort(key=lambda p: -p[2])
        bs = pieces_b[:NCORES]
        ds = pieces_b[NCORES:]
        a = min(c0, A)
        b = bs[0][2] if bs else 0
        c = min(c8, C)
        d = ds[0][2] if ds else 0
        caps = (a, b, c, d)
        # piece -> core assignment avoiding same expert twice on one core
        placement = [[None] * 4 for _ in range(NCORES)]
        for i in range(NCORES):
            placement[i][0] = (heavy[i], 0, min(int(counts[heavy[i]]), A))
            placement[i][2] = (light[i], 0, min(int(counts[light[i]]), C))
        for sl, plist in ((1, bs), (3, ds)):
            free = set(range(NCORES))
            for e, st, ln in plist:
                cand = [i for i in free
                        if e != placement[i][0][0] and e != placement[i][2][0]
                        and (placement[i][1] is None or
                             placement[i][1][0] != e)]
                if not cand:
                    return None
                i = cand[0]
                free.discard(i)
                placement[i][sl] = (e, st, ln)
        return caps, placement

    best = None
    lo_a = (c0 + 1) // 2
    lo_c = (c8 + 1) // 2
    cands = [(c0, c8)]
    for A in range(lo_a, c0 + 1, 2):
        for C in range(lo_c, c8 + 1, 2):
            cands.append((A, C))
    for A, C in cands:
        got = build(A, C)
        if got is None:
            continue
        caps, placement = got
        cost = plan_cost(caps)
        if best is None or cost < best[0]:
            best = (cost, caps, placement)
    _, caps, placement = best
    # drop zero-cap slots; fill empty kept slots with a zero-length piece of
    # some expert not already used by that core (perm needs distinct experts)
    keep = [si for si in range(4) if caps[si] > 0]
    caps_k = tuple(caps[si] for si in keep)
    placement_k = []
    for pl in placement:
        row = []
        used = {p[0] for p in pl if p is not None}
        for si in keep:
            p = pl[si]
            if p is None:
                e_fill = next(e for e in range(E_) if e not in used)
                used.add(e_fill)
                p = (e_fill, 0, 0)
            row.append(p)
        placement_k.append(row)
    return caps_k, placement_k


def _chunk_sizes(cap: int, rem_first: bool) -> list[int]:
    """Split cap into <=512-sized chunks; remainder first or last."""
    n_full, rem = divmod(cap, 512)
    sizes = [512] * n_full
    if rem:
        if rem_first:
            sizes = [rem] + sizes
        else:
            sizes = sizes + [rem]
    return sizes


def _build_moe(caps: tuple[int, ...]):
    """One-launch MoE FFN + on-device combine weights.

    Inputs per core (S = len(caps) expert slots):
      w13t  [S, H, 2F] bf16  per-slot hstack(w1[e].T, w3[e].T)
      w2t   [S, F, H]  bf16  per-slot w2[e].T
      xgt   [H, CT]    bf16  gathered tokens (transposed), CT = sum(caps)
      gtp   [H, E]     bf16  gate_w.T, columns permuted so that column s is
                             slot s's expert
      biasp [128, E]   f32   e_score_correction_bias, same permutation,
                             broadcast to 128 partitions
    Output:
      yg    [CT, H]    bf16  combine-weighted expert outputs per gathered token
    """
    S = len(caps)
    CT = sum(caps)
    chunk_lists = [_chunk_sizes(cap, rem_first=False) for cap in caps]
    ntiles_total = sum(math.ceil(tl / 128)
                       for chunks in chunk_lists for tl in chunks)

    nc = bacc.Bacc("TRN2", target_bir_lowering=False, debug=False,
                   num_devices=NCORES)
    w13 = nc.dram_tensor("w13t", [S, H, 2 * F], BF16,
                         kind="ExternalInput").ap()
    w2t = nc.dram_tensor("w2t", [S, F, H], BF16, kind="ExternalInput").ap()
    xgt = nc.dram_tensor("xgt", [H, CT], BF16, kind="ExternalInput").ap()
    gtp = nc.dram_tensor("gtp", [H, E], BF16, kind="ExternalInput").ap()
    biasp = nc.dram_tensor("biasp", [128, E], F32, kind="ExternalInput").ap()
    yg = nc.dram_tensor("yg", [CT, H], BF16, kind="ExternalOutput").ap()

    xgt_r = xgt.rearrange("(ko p) t -> p ko t", p=128)
    SIG = mybir.ActivationFunctionType.Sigmoid

    with tile.TileContext(nc) as tc:
        with (
            tc.tile_pool(name="const_p", bufs=1) as const_p,
            tc.tile_pool(name="w13_p", bufs=2) as w13_p,
            tc.tile_pool(name="w2_p", bufs=2) as w2_p,
            tc.tile_pool(name="xg_p", bufs=3) as xg_p,
            tc.tile_pool(name="ht_p", bufs=2) as ht_p,
            tc.tile_pool(name="sg_p", bufs=2) as sg_p,
            tc.tile_pool(name="y_p", bufs=3) as y_p,
            tc.tile_pool(name="work_p", bufs=2) as work_p,
            tc.tile_pool(name="ps", bufs=4, space="PSUM") as ps_pool,
        ):
            gt_sb = const_p.tile([128, KC, E], BF16)
            bias_sb = const_p.tile([128, E], F32)
            w_sb = const_p.tile([128, ntiles_total], F32)

            def routing(xg_sb, tl, nt, s, ci, jglob):
                """Combine weight of this slot's expert for one token chunk."""
                ps_r = ps_pool.tile([128, nt, E], F32, tag="psr",
                                    bufs=2, name=f"ps_r_{s}_{ci}")
                # partial last tile leaves rows >= ttl unwritten by the
                # matmuls; zero-fill so batched reads are fully defined
                nc.vector.memset(ps_r[:, :nt, :], 0.0)
                for j in range(nt):
                    tt0 = j * 128
                    ttl = min(128, tl - tt0)
                    for k in range(KC):
                        nc.tensor.matmul(
                            ps_r[:ttl, j, :],
                            lhsT=xg_sb[k][:, tt0:tt0 + ttl],
                            rhs=gt_sb[:, k, :],
                            start=(k == 0), stop=(k == KC - 1))
                sc = work_p.tile([128, nt, E], F32, tag="sc",
                                 name=f"sc_{s}_{ci}", padded_shape=[128, 4, E])
                nc.scalar.activation(sc[:, :nt, :], ps_r[:, :nt, :], SIG)
                biased = work_p.tile([128, nt, E], F32, tag="biased",
                                     name=f"biased_{s}_{ci}",
                                     padded_shape=[128, 4, E])
                nc.vector.tensor_tensor(
                    biased[:, :nt, :], sc[:, :nt, :],
                    bias_sb[:, None, :].to_broadcast([128, nt, E]),
                    mybir.AluOpType.add)
                m8 = work_p.tile([128, nt, 8], F32, tag="m8",
                                 name=f"m8_{s}_{ci}", padded_shape=[128, 4, 8])
                sel = work_p.tile([128, nt, E], F32, tag="sel",
                                  name=f"sel_{s}_{ci}",
                                  padded_shape=[128, 4, E])
                for j in range(nt):
                    nc.vector.max(m8[:, j, :], biased[:, j, :])
                for j in range(nt):
                    nc.vector.tensor_scalar(
                        sel[:, j, :], biased[:, j, :],
                        m8[:, j, TOPK - 1:TOPK], None,
                        op0=mybir.AluOpType.is_ge)
                picked = work_p.tile([128, nt, E], F32, tag="picked",
                                     name=f"picked_{s}_{ci}",
                                     padded_shape=[128, 4, E])
                nc.vector.tensor_mul(
                    picked[:, :nt, :], sel[:, :nt, :], sc[:, :nt, :])
                denom = work_p.tile([128, nt], F32, tag="denom",
                                    name=f"denom_{s}_{ci}",
                                    padded_shape=[128, 4])
                nc.vector.reduce_sum(
                    denom[:, :nt], picked[:, :nt, :], axis=mybir.AxisListType.X)
                recip = work_p.tile([128, nt], F32, tag="recip",
                                    name=f"recip_{s}_{ci}",
                                    padded_shape=[128, 4])
                nc.vector.reciprocal(recip[:, :nt], denom[:, :nt])
                # slot expert score is column s (host permutation)
                nc.vector.tensor_mul(
                    w_sb[:, jglob:jglob + nt], sc[:, :nt, s], recip[:, :nt])

            def evac_stage1(ps_g, ps_u, ht_sb, fi, tl):
                """ht[:, fi, :tl] = silu(g) * u = g * sigmoid(g) * u."""
                sgm = sg_p.tile([128, 512], F32, tag="sgm", name=f"sgm_{fi}")
                nc.scalar.activation(sgm[:, :tl], ps_g[:, :tl], SIG)
                gsg = sg_p.tile([128, 512], F32, tag="gsg", name=f"gsg_{fi}")
                nc.vector.tensor_mul(gsg[:, :tl], sgm[:, :tl], ps_g[:, :tl])
                nc.vector.tensor_mul(
                    ht_sb[:, fi, :tl], gsg[:, :tl], ps_u[:, :tl])

            def emit_stage2(p, final):
                """Stage 2 for one chunk: y[t,h] = w[t]*sum_f hT[f,t]*w2T[f,h].

                Emitted one chunk late (software pipeline) so the PE never
                waits on the chunk's own ht evacuation chain.
                """
                tl, nt, t0 = p["tl"], p["nt"], p["t0"]
                ht_sb, w2_sb, jg = p["ht_sb"], p["w2_sb"], p["jglob"]
                for j in range(nt):
                    tt0 = j * 128
                    ttl = min(128, tl - tt0)
                    wj = w_sb[:ttl, jg + j:jg + j + 1]
                    y_sb = y_p.tile([128, H], BF16, tag="y", name=f"y_sb_{jg+j}")
                    ps_ys = []
                    for hh in range(2):
                        ps_y = ps_pool.tile([128, 512], F32, tag="psy",
                                            bufs=3, name=f"ps_y_{jg+j}_{hh}")
                        ps_ys.append(ps_y)
                        for kf in range(FC):
                            nc.tensor.matmul(
                                ps_y[:ttl],
                                lhsT=ht_sb[:, kf, tt0:tt0 + ttl],
                                rhs=w2_sb[:, kf, hh * 512:(hh + 1) * 512],
                                start=(kf == 0), stop=(kf == FC - 1))
                        if hh == 0 or not (final and j == nt - 1):
                            nc.vector.tensor_scalar(
                                y_sb[:ttl, hh * 512:(hh + 1) * 512],
                                ps_y[:ttl], wj, None,
                                op0=mybir.AluOpType.mult)
                    if final and j == nt - 1:
                        # final tile: drain the second half as two quarter
                        # pieces on parallel engines/queues to shorten the
                        # end-of-kernel DMA latency chain
                        rows = slice(t0 + tt0, t0 + tt0 + ttl)
                        nc.sync.dma_start(
                            yg[rows, 0:512], y_sb[:ttl, 0:512])
                        nc.scalar.activation(
                            y_sb[:ttl, 512:768], ps_ys[1][:ttl, 0:256],
                            mybir.ActivationFunctionType.Copy,
                            scale=wj)
                        nc.scalar.dma_start(
                            yg[rows, 512:768], y_sb[:ttl, 512:768])
                        ybq = y_p.tile([128, 256], BF16, tag="ybq",
                                       bufs=1, name="ybq_last")
                        nc.vector.tensor_scalar(
                            ybq[:ttl, :], ps_ys[1][:ttl, 256:512],
                            wj, None, op0=mybir.AluOpType.mult)
                        nc.sync.dma_start(
                            yg[rows, 768:1024], ybq[:ttl, :])
                    else:
                        nc.sync.dma_start(
                            yg[t0 + tt0:t0 + tt0 + ttl, :], y_sb[:ttl, :])

            jglob = 0
            pending = None
            for s in range(S):
                cap = caps[s]
                off = sum(caps[:s])
                chunks = chunk_lists[s]

                # k=0 weights split into g/u halves so the first matmul's DMA
                # dependency is small; k>=1 combined to halve the issue count
                w13g0 = w13_p.tile([128, F], BF16, tag="w13g0",
                                   name=f"w13g0_{s}")
                w13u0 = w13_p.tile([128, F], BF16, tag="w13u0",
                                   name=f"w13u0_{s}")
                nc.sync.dma_start(w13g0[:], w13[s, 0:128, 0:F])
                nc.sync.dma_start(w13u0[:], w13[s, 0:128, F:2 * F])
                w13k = [None] + [w13_p.tile([128, 2 * F], BF16,
                                            tag=f"w13_{k}",
                                            name=f"w13_sb_{s}_{k}")
                                 for k in range(1, KC)]
                for k in range(1, KC):
                    nc.sync.dma_start(
                        w13k[k][:], w13[s, k * 128:(k + 1) * 128, :])

                def gv(k, fi):
                    if k == 0:
                        return w13g0[:, fi * 128:(fi + 1) * 128]
                    return w13k[k][:, fi * 128:(fi + 1) * 128]

                def uv(k, fi):
                    if k == 0:
                        return w13u0[:, fi * 128:(fi + 1) * 128]
                    return w13k[k][:, F + fi * 128:F + (fi + 1) * 128]

                tch0 = 0
                for ci, tl in enumerate(chunks):
                    t0 = off + tch0
                    tch0 += tl
                    nt = math.ceil(tl / 128)
                    ramp = (s == 0 and ci == 0)

                    xg_big = xg_p.tile([128, KC, 512], BF16, tag="xg",
                                       name=f"xg_sb_{s}_{ci}")
                    xg_sb = [xg_big[:, k, :] for k in range(KC)]
                    if ramp:
                        # per-k DMAs so the PE can consume k-chunks as they
                        # stream in during the cold start
                        for k in range(KC):
                            nc.scalar.dma_start(
                                xg_big[:, k, :tl], xgt_r[:, k, t0:t0 + tl])
                    else:
                        nc.sync.dma_start(
                            xg_big[:, :, :tl], xgt_r[:, :, t0:t0 + tl])
                    if ramp:
                        # routing consts + slot-0 w2 AFTER the ramp-critical
                        # xg tiles (a big early w2 transfer would stall the
                        # first matmuls behind it on the shared DMA engines)
                        nc.scalar.dma_start(
                            gt_sb[:],
                            gtp.rearrange("(ko p) e -> p ko e", p=128))
                        nc.scalar.dma_start(bias_sb[:], biasp[:])
                    if ci == 0:
                        w2_sb = w2_p.tile([128, FC, H], BF16, tag="w2",
                                          name=f"w2_sb_{s}")
                        nc.scalar.dma_start(
                            w2_sb[:],
                            w2t[s].rearrange("(ko p) h -> p ko h", p=128))

                    ht_sb = ht_p.tile([128, FC, 512], BF16, tag="ht")

                    if not ramp:
                        # routing first: its ACT+DVE chain then completes
                        # during stage 1, well before stage 2 consumes w_sb
                        routing(xg_sb, tl, nt, s, ci, jglob)

                    # ---- stage 1: hT[f,t] = silu(x@w1.T).T * (x@w3.T).T ----
                    if ramp:
                        # k OUTER across all fi: the PE consumes each
                        # weight/activation k-chunk as it streams in.
                        # 8 live PSUM tiles across the three tags.
                        tags = ["ps1", "ps1", "ps1", "psy",
                                "psy", "psy", "psr", "psr"]
                        ps8 = [ps_pool.tile([128, 512], F32, tag=tags[i],
                                            bufs=(2 if tags[i] == "psr"
                                                  else 3),
                                            name=f"ps_ramp_{i}")
                               for i in range(8)]
                        ps_gs = ps8[0::2]
                        ps_us = ps8[1::2]
                        for k in range(KC):
                            for fi in range(FC):
                                nc.tensor.matmul(
                                    ps_gs[fi][:, :tl], lhsT=gv(k, fi),
                                    rhs=xg_sb[k][:, :tl],
                                    start=(k == 0), stop=(k == KC - 1))
                                nc.tensor.matmul(
                                    ps_us[fi][:, :tl], lhsT=uv(k, fi),
                                    rhs=xg_sb[k][:, :tl],
                                    start=(k == 0), stop=(k == KC - 1))
                        for fi in range(FC):
                            evac_stage1(ps_gs[fi], ps_us[fi], ht_sb, fi, tl)
                        # ramp routing last (needs every xg k-chunk anyway)
                        routing(xg_sb, tl, nt, s, ci, jglob)
                    else:
                        # fi sequential, k inner: only 2 PSUM tiles live
                        for fi in range(FC):
                            ps_g = ps_pool.tile([128, 512], F32, tag="ps1",
                                                bufs=3,
                                                name=f"ps_g_{s}_{ci}_{fi}")
                            ps_u = ps_pool.tile([128, 512], F32, tag="ps1",
                                                bufs=3,
                                                name=f"ps_u_{s}_{ci}_{fi}")
                            for k in range(KC):
                                nc.tensor.matmul(
                                    ps_g[:, :tl], lhsT=gv(k, fi),
                                    rhs=xg_sb[k][:, :tl],
                                    start=(k == 0), stop=(k == KC - 1))
                                nc.tensor.matmul(
                                    ps_u[:, :tl], lhsT=uv(k, fi),
                                    rhs=xg_sb[k][:, :tl],
                                    start=(k == 0), stop=(k == KC - 1))
                            evac_stage1(ps_g, ps_u, ht_sb, fi, tl)

                    # ---- stage 2 of the PREVIOUS chunk (pipelined) ----
                    if pending is not None:
                        emit_stage2(pending, final=False)
                    pending = {"tl": tl, "nt": nt, "t0": t0, "ht_sb": ht_sb,
                               "w2_sb": w2_sb, "jglob": jglob}
                    jglob += nt

            emit_stage2(pending, final=True)

    nc.compile()
    return nc


def _moe_nc(caps):
    key = ("moe", caps)
    if key not in _nc_cache:
        _nc_cache[key] = _build_moe(caps)
    return _nc_cache[key]


def kernel(hidden_states, gate_w, bias, w1, w3, w2):
    x = np.ascontiguousarray(np.asarray(hidden_states, dtype=np.float32))
    gate_w = np.asarray(gate_w, dtype=np.float32)
    bias = np.asarray(bias, dtype=np.float32)
    w1 = np.asarray(w1, dtype=np.float32)
    w3 = np.asarray(w3, dtype=np.float32)
    w2 = np.asarray(w2, dtype=np.float32)

    # ---- Host dispatch: fp32 routing decides token->expert placement ----
    logits = x @ gate_w.T                                # [T, E]
    scores = 1.0 / (1.0 + np.exp(-logits))
    biased = scores + bias[None, :]
    topi = np.argpartition(-biased, TOPK - 1, axis=1)[:, :TOPK]  # [T, K] sets
    sel = np.zeros((T, E), dtype=bool)
    sel[np.arange(T)[:, None], topi] = True
    idx_per_e = [np.nonzero(sel[:, e])[0] for e in range(E)]
    counts = np.array([len(ix) for ix in idx_per_e])
    caps, placement = _plan_slots(counts)
    S = len(caps)
    offs = [sum(caps[:si]) for si in range(S)]
    global LAST_CAPS
    LAST_CAPS = caps
    CT = sum(caps)

    xT = np.ascontiguousarray(x.T)                       # [H, T]
    xT16 = xT.astype(ml_dtypes.bfloat16)
    gT16 = np.ascontiguousarray(gate_w.T).astype(ml_dtypes.bfloat16)

    in_maps = []
    for c in range(NCORES):
        slot_experts = [p[0] for p in placement[c]]
        idx_pad = np.zeros(CT, dtype=np.int64)
        for si, (e, st, ln) in enumerate(placement[c]):
            if ln:
                idx_pad[offs[si]:offs[si] + ln] = idx_per_e[e][st:st + ln]
        xgt = np.ascontiguousarray(xT16[:, idx_pad])     # [H, CT] bf16
        w13t = np.stack([
            np.ascontiguousarray(
                np.concatenate([w1[e].T, w3[e].T], axis=1))
            for e in slot_experts]).astype(ml_dtypes.bfloat16)  # [S, H, 2F]
        w2t = np.stack(
            [np.ascontiguousarray(w2[e].T) for e in slot_experts]
        ).astype(ml_dtypes.bfloat16)
        perm = slot_experts + [e for e in range(E) if e not in slot_experts]
        gtp = np.ascontiguousarray(gT16[:, perm])        # [H, E] bf16
        biasp = np.ascontiguousarray(
            np.broadcast_to(np.asarray(bias)[perm][None, :],
                            (128, E))).astype(np.float32)
        in_maps.append(
            {"w13t": w13t, "w2t": w2t, "xgt": xgt, "gtp": gtp,
             "biasp": biasp})

    # ---- Single SPMD launch: routing weights + expert FFN ----
    ncB = _moe_nc(caps)
    res = run_bass_kernel_spmd(ncB, in_maps, core_ids=list(range(NCORES)))

    # ---- Host combine: scatter-add ----
    out = np.zeros((T, H), dtype=np.float32)
    for c in range(NCORES):
        for si, (e, st, ln) in enumerate(placement[c]):
            if ln:
                ix = idx_per_e[e][st:st + ln]
                out[ix] += res.results[c]["yg"][offs[si]:offs[si] + ln
                                                ].astype(np.float32)
    return out


# revision 30
# speedup vs baseline: 1.0272x; 1.0272x over previous
"""MiniMax-M2 MoE kernel for 8 Trainium2 NeuronCores.

Single-launch expert-parallel design:
  Host (data movement / dispatch only): fp32 routing decides WHICH tokens go
    to WHICH expert (indices only); a planner cuts each expert's token list
    into at most two pieces and packs them into up to 4 static expert slots
    per core (capacities chosen to minimize modeled PE time, ~7% over the
    perfect-balance floor); tokens are gathered per slot and weights
    pre-transposed/cast to bf16.
  Device (all output-value arithmetic, one SPMD launch):
    - per slot, recompute router scores for the slot's gathered tokens
      (logits -> sigmoid -> top-4 threshold on bias-corrected scores ->
      renormalized combine weight of the slot's own expert; the host permutes
      the gate matrix per core so slot s's expert is always column s),
    - SwiGLU FFN (bf16 matmuls) and combine-weight scaling.  silu(g) is
      computed as g * sigmoid(g) so the Activation engine only ever needs the
      sigmoid table (one LoadActFuncSet instead of thrashing Silu<->Sigmoid).
    - stage 2 runs one chunk behind stage 1 (software pipeline) so the PE
      never idles on the ht evacuation chain.
  Host: scatter-add per-slot outputs into [T, H].
"""

import math

import ml_dtypes
import numpy as np

import concourse.bass as bass
import concourse.tile as tile
from concourse import bacc, mybir
from concourse.bass_utils import run_bass_kernel_spmd

T, H, F, E, TOPK = 4096, 1024, 512, 16, 4
NCORES = 8
KC = H // 128   # contraction chunks (hidden dim)
FC = F // 128   # stage-2 contraction chunks
F32 = mybir.dt.float32
BF16 = mybir.dt.bfloat16

_nc_cache: dict = {}
LAST_CAPS = (832, 492, 512, 354)  # caps used by the most recent kernel() call


# Good general cap vectors found by offline search on the canonical routing
# distribution; each is validated against the ACTUAL counts at runtime (DP
# feasibility + placement construction) before use.
_CAPS_CANDIDATES = [(684, 604, 460, 350)]


def _dp_assign(caps, counts_desc):
    """Assign each expert (counts desc) a pair of slot types (i<=j) such
    that caps[i]+caps[j] >= count and each type is used at most 8 times.
    Returns the choice list or None."""
    pairs = [(i, j) for i in range(len(caps)) for j in range(i, len(caps))]
    capsum = {p: caps[p[0]] + caps[p[1]] for p in pairs}
    opts = []
    for c in counts_desc:
        o = [p for p in pairs if capsum[p] >= c]
        if not o:
            return None
        opts.append(o)
    n = len(counts_desc)
    seen = set()
    choice = [None] * n

    def dfs(k, rem):
        if k == n:
            return True
        key = (k, rem)
        if key in seen:
            return False
        for (i, j) in opts[k]:
            r2 = list(rem)
            r2[i] -= 1
            r2[j] -= 1
            if r2[i] >= 0 and r2[j] >= 0:
                choice[k] = (i, j)
                if dfs(k + 1, tuple(r2)):
                    return True
        seen.add(key)
        return False

    if not dfs(0, (NCORES,) * len(caps)):
        return None
    return choice


def _place_from_choice(caps, experts_desc, counts, choice):
    """Build placement[core][slot] = (expert, tok_start, len) from a
    type-pair assignment; both pieces of one expert land on distinct cores.
    Returns placement or None."""
    S = len(caps)
    pieces_per_type = [[] for _ in range(S)]
    for k, e in enumerate(experts_desc):
        i, j = choice[k]
        c = int(counts[e])
        pi = min(caps[i], c)
        pj = c - pi
        pieces_per_type[i].append((e, 0, pi))
        pieces_per_type[j].append((e, pi, pj))
    for rot in range(NCORES):
        placement = [[None] * S for _ in range(NCORES)]
        ok = True
        for t in range(S):
            free = list(range(NCORES))
            free = free[rot:] + free[:rot]
            for (e, st, ln) in sorted(pieces_per_type[t],
                                      key=lambda p: -p[2]):
                cand = [ci for ci in free
                        if e not in {p[0] for p in placement[ci] if p}]
                if not cand:
                    ok = False
                    break
                ci = cand[0]
                placement[ci][t] = (e, st, ln)
                free.remove(ci)
            if not ok:
                break
        if ok:
            return placement
    return None


def _plan_slots(counts: np.ndarray):
    """Choose per-core slot capacities and expert-piece placement.

    Experts are cut into at most two pieces assigned to a pair of slot
    types.  First the precomputed general cap vectors are tried (exact DP
    feasibility on the actual counts); otherwise a threshold-cut search
    (heavy/light primaries + ranked remainders) provides the fallback.
    Cost model: stage-1 PE time scales with total capacity, stage-2/routing
    with ceil(cap/128) tiles.

    Returns (caps, placement) where placement[core] is a list of
    (expert, tok_start, length) per slot (length may be 0).
    """
    E_ = len(counts)
    order = np.argsort(-counts, kind="stable")
    heavy = [int(e) for e in order[:NCORES]]
    light = [int(e) for e in order[NCORES:]]
    c0 = int(counts[heavy[0]])
    c8 = int(counts[light[0]])

    def plan_cost(caps):
        ct = sum(caps)
        tiles = sum(math.ceil(cp / 128) for cp in caps if cp)
        return 64 * ct + (8 * 512 + 8 * E_) * tiles

    def build(A, C):
        pieces_b = []  # (expert, start, len) remainders
        for e in heavy:
            if counts[e] > A:
                pieces_b.append((e, A, int(counts[e]) - A))
        for e in light:
            if counts[e] > C:
                pieces_b.append((e, C, int(counts[e]) - C))
        if len(pieces_b) > 2 * NCORES:
            return None
        pieces_b.sort(key=lambda p: -p[2])
        bs = pieces_b[:NCORES]
        ds = pieces_b[NCORES:]
        a = min(c0, A)
        b = bs[0][2] if bs else 0
        c = min(c8, C)
        d = ds[0][2] if ds else 0
        caps = (a, b, c, d)
        # piece -> core assignment avoiding same expert twice on one core
        placement = [[None] * 4 for _ in range(NCORES)]
        for i in range(NCORES):
            placement[i][0] = (heavy[i], 0, min(int(counts[heavy[i]]), A))
            placement[i][2] = (light[i], 0, min(int(counts[light[i]]), C))
        for sl, plist in ((1, bs), (3, ds)):
            free = set(range(NCORES))
            for e, st, ln in plist:
                cand = [i for i in free
                        if e != placement[i][0][0] and e != placement[i][2][0]
                        and (placement[i][1] is None or
                             placement[i][1][0] != e)]
                if not cand:
                    return None
                i = cand[0]
                free.discard(i)
                placement[i][sl] = (e, st, ln)
        return caps, placement

    best = None
    lo_a = (c0 + 1) // 2
    lo_c = (c8 + 1) // 2
    cands = [(c0, c8)]
    for A in range(lo_a, c0 + 1, 2):
        for C in range(lo_c, c8 + 1, 2):
            cands.append((A, C))
    for A, C in cands:
        got = build(A, C)
        if got is None:
            continue
        caps, placement = got
        cost = plan_cost(caps)
        if best is None or cost < best[0]:
            best = (cost, caps, placement)
    # precomputed general cap vectors (validated against actual counts)
    experts_desc = [int(e) for e in order]
    counts_desc = [int(counts[e]) for e in experts_desc]
    for caps_c in _CAPS_CANDIDATES:
        if plan_cost(caps_c) >= best[0]:
            continue
        choice = _dp_assign(caps_c, counts_desc)
        if choice is None:
            continue
        pl = _place_from_choice(caps_c, experts_desc, counts, choice)
        if pl is None:
            continue
        best = (plan_cost(caps_c), caps_c, pl)
    _, caps, placement = best
    # drop zero-cap slots; fill empty kept slots with a zero-length piece of
    # some expert not already used by that core (perm needs distinct experts)
    keep = [si for si in range(4) if caps[si] > 0]
    caps_k = tuple(caps[si] for si in keep)
    placement_k = []
    for pl in placement:
        row = []
        used = {p[0] for p in pl if p is not None}
        for si in keep:
            p = pl[si]
            if p is None:
                e_fill = next(e for e in range(E_) if e not in used)
                used.add(e_fill)
                p = (e_fill, 0, 0)
            row.append(p)
        placement_k.append(row)
    return caps_k, placement_k


def _chunk_sizes(cap: int, rem_first: bool) -> list[int]:
    """Split cap into <=512-sized chunks; remainder first or last."""
    n_full, rem = divmod(cap, 512)
    sizes = [512] * n_full
    if rem:
        if rem_first:
            sizes = [rem] + sizes
        else:
            sizes = sizes + [rem]
    return sizes


def _build_moe(caps: tuple[int, ...]):
    """One-launch MoE FFN + on-device combine weights.

    Inputs per core (S = len(caps) expert slots):
      w13t  [S, H, 2F] bf16  per-slot hstack(w1[e].T, w3[e].T)
      w2t   [S, F, H]  bf16  per-slot w2[e].T
      xgt   [H, CT]    bf16  gathered tokens (transposed), CT = sum(caps)
      gtp   [H, E]     bf16  gate_w.T, columns permuted so that column s is
                             slot s's expert
      biasp [128, E]   f32   e_score_correction_bias, same permutation,
                             broadcast to 128 partitions
    Output:
      yg    [CT, H]    bf16  combine-weighted expert outputs per gathered token
    """
    S = len(caps)
    CT = sum(caps)
    chunk_lists = [_chunk_sizes(cap, rem_first=False) for cap in caps]
    ntiles_total = sum(math.ceil(tl / 128)
                       for chunks in chunk_lists for tl in chunks)

    nc = bacc.Bacc("TRN2", target_bir_lowering=False, debug=False,
                   num_devices=NCORES)
    w13 = nc.dram_tensor("w13t", [S, H, 2 * F], BF16,
                         kind="ExternalInput").ap()
    w2t = nc.dram_tensor("w2t", [S, F, H], BF16, kind="ExternalInput").ap()
    xgt = nc.dram_tensor("xgt", [H, CT], BF16, kind="ExternalInput").ap()
    gtp = nc.dram_tensor("gtp", [H, E], BF16, kind="ExternalInput").ap()
    biasp = nc.dram_tensor("biasp", [128, E], F32, kind="ExternalInput").ap()
    yg = nc.dram_tensor("yg", [CT, H], BF16, kind="ExternalOutput").ap()

    xgt_r = xgt.rearrange("(ko p) t -> p ko t", p=128)
    SIG = mybir.ActivationFunctionType.Sigmoid

    with tile.TileContext(nc) as tc:
        with (
            tc.tile_pool(name="const_p", bufs=1) as const_p,
            tc.tile_pool(name="w13_p", bufs=2) as w13_p,
            tc.tile_pool(name="w2_p", bufs=2) as w2_p,
            tc.tile_pool(name="xg_p", bufs=3) as xg_p,
            tc.tile_pool(name="ht_p", bufs=2) as ht_p,
            tc.tile_pool(name="sg_p", bufs=2) as sg_p,
            tc.tile_pool(name="y_p", bufs=3) as y_p,
            tc.tile_pool(name="work_p", bufs=2) as work_p,
            tc.tile_pool(name="ps", bufs=4, space="PSUM") as ps_pool,
        ):
            gt_sb = const_p.tile([128, KC, E], BF16)
            bias_sb = const_p.tile([128, E], F32)
            w_sb = const_p.tile([128, ntiles_total], F32)

            def routing(xg_sb, tl, nt, s, ci, jglob):
                """Combine weight of this slot's expert for one token chunk."""
                ps_r = ps_pool.tile([128, nt, E], F32, tag="psr",
                                    bufs=2, name=f"ps_r_{s}_{ci}")
                # partial last tile leaves rows >= ttl unwritten by the
                # matmuls; zero-fill so batched reads are fully defined
                nc.vector.memset(ps_r[:, :nt, :], 0.0)
                for j in range(nt):
                    tt0 = j * 128
                    ttl = min(128, tl - tt0)
                    for k in range(KC):
                        nc.tensor.matmul(
                            ps_r[:ttl, j, :],
                            lhsT=xg_sb[k][:, tt0:tt0 + ttl],
                            rhs=gt_sb[:, k, :],
                            start=(k == 0), stop=(k == KC - 1))
                # scores = sigmoid(logits) = 0.5 + 0.5*tanh(logits/2); Tanh
                # shares the activation table with Silu (no table reloads)
                th = work_p.tile([128, nt, E], F32, tag="th",
                                 name=f"th_{s}_{ci}", padded_shape=[128, 4, E])
                nc.scalar.activation(
                    th[:, :nt, :], ps_r[:, :nt, :],
                    mybir.ActivationFunctionType.Tanh, scale=0.5)
                sc = work_p.tile([128, nt, E], F32, tag="sc",
                                 name=f"sc_{s}_{ci}", padded_shape=[128, 4, E])
                nc.vector.tensor_scalar(
                    sc[:, :nt, :], th[:, :nt, :], 0.5, 0.5,
                    op0=mybir.AluOpType.mult, op1=mybir.AluOpType.add)
                biased = work_p.tile([128, nt, E], F32, tag="biased",
                                     name=f"biased_{s}_{ci}",
                                     padded_shape=[128, 4, E])
                nc.vector.tensor_tensor(
                    biased[:, :nt, :], sc[:, :nt, :],
                    bias_sb[:, None, :].to_broadcast([128, nt, E]),
                    mybir.AluOpType.add)
                m8 = work_p.tile([128, nt, 8], F32, tag="m8",
                                 name=f"m8_{s}_{ci}", padded_shape=[128, 4, 8])
                sel = work_p.tile([128, nt, E], F32, tag="sel",
                                  name=f"sel_{s}_{ci}",
                                  padded_shape=[128, 4, E])
                for j in range(nt):
                    nc.vector.max(m8[:, j, :], biased[:, j, :])
                for j in range(nt):
                    nc.vector.tensor_scalar(
                        sel[:, j, :], biased[:, j, :],
                        m8[:, j, TOPK - 1:TOPK], None,
                        op0=mybir.AluOpType.is_ge)
                picked = work_p.tile([128, nt, E], F32, tag="picked",
                                     name=f"picked_{s}_{ci}",
                                     padded_shape=[128, 4, E])
                nc.vector.tensor_mul(
                    picked[:, :nt, :], sel[:, :nt, :], sc[:, :nt, :])
                denom = work_p.tile([128, nt], F32, tag="denom",
                                    name=f"denom_{s}_{ci}",
                                    padded_shape=[128, 4])
                nc.vector.reduce_sum(
                    denom[:, :nt], picked[:, :nt, :], axis=mybir.AxisListType.X)
                recip = work_p.tile([128, nt], F32, tag="recip",
                                    name=f"recip_{s}_{ci}",
                                    padded_shape=[128, 4])
                nc.vector.reciprocal(recip[:, :nt], denom[:, :nt])
                # slot expert score is column s (host permutation)
                nc.vector.tensor_mul(
                    w_sb[:, jglob:jglob + nt], sc[:, :nt, s], recip[:, :nt])

            def evac_stage1(ps_g, ps_u, ht_sb, fi, tl):
                """ht[:, fi, :tl] = silu(g) * u."""
                sg = sg_p.tile([128, 512], F32, tag="sgm", name=f"sg_{fi}")
                nc.scalar.activation(sg[:, :tl], ps_g[:, :tl],
                                     mybir.ActivationFunctionType.Silu)
                nc.vector.tensor_mul(
                    ht_sb[:, fi, :tl], sg[:, :tl], ps_u[:, :tl])

            def emit_stage2(p, final):
                """Stage 2 for one chunk: y[t,h] = w[t]*sum_f hT[f,t]*w2T[f,h].

                Emitted one chunk late (software pipeline) so the PE never
                waits on the chunk's own ht evacuation chain.
                """
                tl, nt, t0 = p["tl"], p["nt"], p["t0"]
                ht_sb, w2_sb, jg = p["ht_sb"], p["w2_sb"], p["jglob"]
                for j in range(nt):
                    tt0 = j * 128
                    ttl = min(128, tl - tt0)
                    wj = w_sb[:ttl, jg + j:jg + j + 1]
                    y_sb = y_p.tile([128, H], BF16, tag="y", name=f"y_sb_{jg+j}")
                    ps_ys = []
                    for hh in range(2):
                        ps_y = ps_pool.tile([128, 512], F32, tag="psy",
                                            bufs=3, name=f"ps_y_{jg+j}_{hh}")
                        ps_ys.append(ps_y)
                        for kf in range(FC):
                            nc.tensor.matmul(
                                ps_y[:ttl],
                                lhsT=ht_sb[:, kf, tt0:tt0 + ttl],
                                rhs=w2_sb[:, kf, hh * 512:(hh + 1) * 512],
                                start=(kf == 0), stop=(kf == FC - 1))
                        if hh == 0 or not (final and j == nt - 1):
                            nc.vector.tensor_scalar(
                                y_sb[:ttl, hh * 512:(hh + 1) * 512],
                                ps_y[:ttl], wj, None,
                                op0=mybir.AluOpType.mult)
                    if final and j == nt - 1:
                        # final tile: drain the second half as two quarter
                        # pieces on parallel engines/queues to shorten the
                        # end-of-kernel DMA latency chain
                        rows = slice(t0 + tt0, t0 + tt0 + ttl)
                        nc.sync.dma_start(
                            yg[rows, 0:512], y_sb[:ttl, 0:512])
                        nc.scalar.activation(
                            y_sb[:ttl, 512:768], ps_ys[1][:ttl, 0:256],
                            mybir.ActivationFunctionType.Copy,
                            scale=wj)
                        nc.scalar.dma_start(
                            yg[rows, 512:768], y_sb[:ttl, 512:768])
                        ybq = y_p.tile([128, 256], BF16, tag="ybq",
                                       bufs=1, name="ybq_last")
                        nc.vector.tensor_scalar(
                            ybq[:ttl, :], ps_ys[1][:ttl, 256:512],
                            wj, None, op0=mybir.AluOpType.mult)
                        nc.sync.dma_start(
                            yg[rows, 768:1024], ybq[:ttl, :])
                    else:
                        nc.sync.dma_start(
                            yg[t0 + tt0:t0 + tt0 + ttl, :], y_sb[:ttl, :])

            jglob = 0
            pending = None
            for s in range(S):
                cap = caps[s]
                off = sum(caps[:s])
                chunks = chunk_lists[s]

                # k=0 weights split into g/u halves so the first matmul's DMA
                # dependency is small; k>=1 combined to halve the issue count
                w13g0 = w13_p.tile([128, F], BF16, tag="w13g0",
                                   name=f"w13g0_{s}")
                w13u0 = w13_p.tile([128, F], BF16, tag="w13u0",
                                   name=f"w13u0_{s}")
                nc.sync.dma_start(w13g0[:], w13[s, 0:128, 0:F])
                nc.sync.dma_start(w13u0[:], w13[s, 0:128, F:2 * F])
                w13k = [None] + [w13_p.tile([128, 2 * F], BF16,
                                            tag=f"w13_{k}",
                                            name=f"w13_sb_{s}_{k}")
                                 for k in range(1, KC)]
                for k in range(1, KC):
                    nc.sync.dma_start(
                        w13k[k][:], w13[s, k * 128:(k + 1) * 128, :])

                def gv(k, fi):
                    if k == 0:
                        return w13g0[:, fi * 128:(fi + 1) * 128]
                    return w13k[k][:, fi * 128:(fi + 1) * 128]

                def uv(k, fi):
                    if k == 0:
                        return w13u0[:, fi * 128:(fi + 1) * 128]
                    return w13k[k][:, F + fi * 128:F + (fi + 1) * 128]

                tch0 = 0
                for ci, tl in enumerate(chunks):
                    t0 = off + tch0
                    tch0 += tl
                    nt = math.ceil(tl / 128)
                    ramp = (s == 0 and ci == 0)

                    xg_big = xg_p.tile([128, KC, 512], BF16, tag="xg",
                                       name=f"xg_sb_{s}_{ci}")
                    xg_sb = [xg_big[:, k, :] for k in range(KC)]
                    if ramp:
                        # per-k DMAs so the PE can consume k-chunks as they
                        # stream in during the cold start
                        for k in range(KC):
                            nc.scalar.dma_start(
                                xg_big[:, k, :tl], xgt_r[:, k, t0:t0 + tl])
                    else:
                        nc.sync.dma_start(
                            xg_big[:, :, :tl], xgt_r[:, :, t0:t0 + tl])
                    if ramp:
                        # routing consts + slot-0 w2 AFTER the ramp-critical
                        # xg tiles (a big early w2 transfer would stall the
                        # first matmuls behind it on the shared DMA engines)
                        nc.scalar.dma_start(
                            gt_sb[:],
                            gtp.rearrange("(ko p) e -> p ko e", p=128))
                        nc.scalar.dma_start(bias_sb[:], biasp[:])
                    if ci == 0:
                        w2_sb = w2_p.tile([128, FC, H], BF16, tag="w2",
                                          name=f"w2_sb_{s}")
                        nc.scalar.dma_start(
                            w2_sb[:],
                            w2t[s].rearrange("(ko p) h -> p ko h", p=128))

                    ht_sb = ht_p.tile([128, FC, 512], BF16, tag="ht")

                    if not ramp:
                        # routing first: its ACT+DVE chain then completes
                        # during stage 1, well before stage 2 consumes w_sb
                        routing(xg_sb, tl, nt, s, ci, jglob)

                    # ---- stage 1: hT[f,t] = silu(x@w1.T).T * (x@w3.T).T ----
                    if ramp:
                        # k OUTER across all fi: the PE consumes each
                        # weight/activation k-chunk as it streams in.
                        # 8 live PSUM tiles across the three tags.
                        tags = ["ps1", "ps1", "ps1", "psy",
                                "psy", "psy", "psr", "psr"]
                        ps8 = [ps_pool.tile([128, 512], F32, tag=tags[i],
                                            bufs=(2 if tags[i] == "psr"
                                                  else 3),
                                            name=f"ps_ramp_{i}")
                               for i in range(8)]
                        ps_gs = ps8[0::2]
                        ps_us = ps8[1::2]
                        for k in range(KC):
                            for fi in range(FC):
                                nc.tensor.matmul(
                                    ps_gs[fi][:, :tl], lhsT=gv(k, fi),
                                    rhs=xg_sb[k][:, :tl],
                                    start=(k == 0), stop=(k == KC - 1))
                                nc.tensor.matmul(
                                    ps_us[fi][:, :tl], lhsT=uv(k, fi),
                                    rhs=xg_sb[k][:, :tl],
                                    start=(k == 0), stop=(k == KC - 1))
                        for fi in range(FC):
                            evac_stage1(ps_gs[fi], ps_us[fi], ht_sb, fi, tl)
                        # ramp routing last (needs every xg k-chunk anyway)
                        routing(xg_sb, tl, nt, s, ci, jglob)
                    else:
                        # fi sequential, k inner: only 2 PSUM tiles live
                        for fi in range(FC):
                            ps_g = ps_pool.tile([128, 512], F32, tag="ps1",
                                                bufs=3,
                                                name=f"ps_g_{s}_{ci}_{fi}")
                            ps_u = ps_pool.tile([128, 512], F32, tag="ps1",
                                                bufs=3,
                                                name=f"ps_u_{s}_{ci}_{fi}")
                            for k in range(KC):
                                nc.tensor.matmul(
                                    ps_g[:, :tl], lhsT=gv(k, fi),
                                    rhs=xg_sb[k][:, :tl],
                                    start=(k == 0), stop=(k == KC - 1))
                                nc.tensor.matmul(
                                    ps_u[:, :tl], lhsT=uv(k, fi),
                                    rhs=xg_sb[k][:, :tl],
                                    start=(k == 0), stop=(k == KC - 1))
                            evac_stage1(ps_g, ps_u, ht_sb, fi, tl)

                    # ---- stage 2 of the PREVIOUS chunk (pipelined) ----
                    if pending is not None:
                        emit_stage2(pending, final=False)
                    pending = {"tl": tl, "nt": nt, "t0": t0, "ht_sb": ht_sb,
                               "w2_sb": w2_sb, "jglob": jglob}
                    jglob += nt

            emit_stage2(pending, final=True)

    nc.compile()
    return nc


def _moe_nc(caps):
    key = ("moe", caps)
    if key not in _nc_cache:
        _nc_cache[key] = _build_moe(caps)
    return _nc_cache[key]


def kernel(hidden_states, gate_w, bias, w1, w3, w2):
    x = np.ascontiguousarray(np.asarray(hidden_states, dtype=np.float32))
    gate_w = np.asarray(gate_w, dtype=np.float32)
    bias = np.asarray(bias, dtype=np.float32)
    w1 = np.asarray(w1, dtype=np.float32)
    w3 = np.asarray(w3, dtype=np.float32)
    w2 = np.asarray(w2, dtype=np.float32)

    # ---- Host dispatch: fp32 routing decides token->expert placement ----
    logits = x @ gate_w.T                                # [T, E]
    scores = 1.0 / (1.0 + np.exp(-logits))
    biased = scores + bias[None, :]
    topi = np.argpartition(-biased, TOPK - 1, axis=1)[:, :TOPK]  # [T, K] sets
    sel = np.zeros((T, E), dtype=bool)
    sel[np.arange(T)[:, None], topi] = True
    idx_per_e = [np.nonzero(sel[:, e])[0] for e in range(E)]
    counts = np.array([len(ix) for ix in idx_per_e])
    caps, placement = _plan_slots(counts)
    S = len(caps)
    offs = [sum(caps[:si]) for si in range(S)]
    global LAST_CAPS
    LAST_CAPS = caps
    CT = sum(caps)

    xT = np.ascontiguousarray(x.T)                       # [H, T]
    xT16 = xT.astype(ml_dtypes.bfloat16)
    gT16 = np.ascontiguousarray(gate_w.T).astype(ml_dtypes.bfloat16)

    in_maps = []
    for c in range(NCORES):
        slot_experts = [p[0] for p in placement[c]]
        idx_pad = np.zeros(CT, dtype=np.int64)
        for si, (e, st, ln) in enumerate(placement[c]):
            if ln:
                idx_pad[offs[si]:offs[si] + ln] = idx_per_e[e][st:st + ln]
        xgt = np.ascontiguousarray(xT16[:, idx_pad])     # [H, CT] bf16
        w13t = np.stack([
            np.ascontiguousarray(
                np.concatenate([w1[e].T, w3[e].T], axis=1))
            for e in slot_experts]).astype(ml_dtypes.bfloat16)  # [S, H, 2F]
        w2t = np.stack(
            [np.ascontiguousarray(w2[e].T) for e in slot_experts]
        ).astype(ml_dtypes.bfloat16)
        perm = slot_experts + [e for e in range(E) if e not in slot_experts]
        gtp = np.ascontiguousarray(gT16[:, perm])        # [H, E] bf16
        biasp = np.ascontiguousarray(
            np.broadcast_to(np.asarray(bias)[perm][None, :],
                            (128, E))).astype(np.float32)
        in_maps.append(
            {"w13t": w13t, "w2t": w2t, "xgt": xgt, "gtp": gtp,
             "biasp": biasp})

    # ---- Single SPMD launch: routing weights + expert FFN ----
    ncB = _moe_nc(caps)
    res = run_bass_kernel_spmd(ncB, in_maps, core_ids=list(range(NCORES)))

    # ---- Host combine: scatter-add ----
    out = np.zeros((T, H), dtype=np.float32)
    for c in range(NCORES):
        for si, (e, st, ln) in enumerate(placement[c]):
            if ln:
                ix = idx_per_e[e][st:st + ln]
                out[ix] += res.results[c]["yg"][offs[si]:offs[si] + ln
                                                ].astype(np.float32)
    return out


# revision 38
# speedup vs baseline: 1.0399x; 1.0123x over previous
"""MiniMax-M2 MoE kernel for 8 Trainium2 NeuronCores.

Single-launch expert-parallel design:
  Host (data movement / dispatch only): fp32 routing decides WHICH tokens go
    to WHICH expert (indices only); a planner cuts each expert's token list
    into at most two pieces and packs them into up to 4 static expert slots
    per core (capacities chosen to minimize modeled PE time, ~7% over the
    perfect-balance floor); tokens are gathered per slot and weights
    pre-transposed/cast to bf16.
  Device (all output-value arithmetic, one SPMD launch):
    - per slot, recompute router scores for the slot's gathered tokens
      (logits -> sigmoid -> top-4 threshold on bias-corrected scores ->
      renormalized combine weight of the slot's own expert; the host permutes
      the gate matrix per core so slot s's expert is always column s),
    - SwiGLU FFN (bf16 matmuls) and combine-weight scaling.  silu(g) is
      computed as g * sigmoid(g) so the Activation engine only ever needs the
      sigmoid table (one LoadActFuncSet instead of thrashing Silu<->Sigmoid).
    - stage 2 runs one chunk behind stage 1 (software pipeline) so the PE
      never idles on the ht evacuation chain.
  Host: scatter-add per-slot outputs into [T, H].
"""

import math

import ml_dtypes
import numpy as np

import concourse.bass as bass
import concourse.tile as tile
from concourse import bacc, mybir
from concourse.bass_utils import run_bass_kernel_spmd

T, H, F, E, TOPK = 4096, 1024, 512, 16, 4
NCORES = 8
KC = H // 128   # contraction chunks (hidden dim)
FC = F // 128   # stage-2 contraction chunks
F32 = mybir.dt.float32
BF16 = mybir.dt.bfloat16

_nc_cache: dict = {}
LAST_CAPS = (832, 492, 512, 354)  # caps used by the most recent kernel() call


# Good general cap vectors found by offline search on the canonical routing
# distribution; each is validated against the ACTUAL counts at runtime (DP
# feasibility + placement construction) before use.
_CAPS_CANDIDATES = [(684, 604, 460, 350)]


def _dp_assign(caps, counts_desc):
    """Assign each expert (counts desc) a pair of slot types (i<=j) such
    that caps[i]+caps[j] >= count and each type is used at most 8 times.
    Returns the choice list or None."""
    pairs = [(i, j) for i in range(len(caps)) for j in range(i, len(caps))]
    capsum = {p: caps[p[0]] + caps[p[1]] for p in pairs}
    opts = []
    for c in counts_desc:
        o = [p for p in pairs if capsum[p] >= c]
        if not o:
            return None
        opts.append(o)
    n = len(counts_desc)
    seen = set()
    choice = [None] * n

    def dfs(k, rem):
        if k == n:
            return True
        key = (k, rem)
        if key in seen:
            return False
        for (i, j) in opts[k]:
            r2 = list(rem)
            r2[i] -= 1
            r2[j] -= 1
            if r2[i] >= 0 and r2[j] >= 0:
                choice[k] = (i, j)
                if dfs(k + 1, tuple(r2)):
                    return True
        seen.add(key)
        return False

    if not dfs(0, (NCORES,) * len(caps)):
        return None
    return choice


def _place_from_choice(caps, experts_desc, counts, choice):
    """Build placement[core][slot] = (expert, tok_start, len) from a
    type-pair assignment; both pieces of one expert land on distinct cores.
    Returns placement or None."""
    S = len(caps)
    pieces_per_type = [[] for _ in range(S)]
    for k, e in enumerate(experts_desc):
        i, j = choice[k]
        c = int(counts[e])
        pi = min(caps[i], c)
        pj = c - pi
        pieces_per_type[i].append((e, 0, pi))
        pieces_per_type[j].append((e, pi, pj))
    for rot in range(NCORES):
        placement = [[None] * S for _ in range(NCORES)]
        ok = True
        for t in range(S):
            free = list(range(NCORES))
            free = free[rot:] + free[:rot]
            for (e, st, ln) in sorted(pieces_per_type[t],
                                      key=lambda p: -p[2]):
                cand = [ci for ci in free
                        if e not in {p[0] for p in placement[ci] if p}]
                if not cand:
                    ok = False
                    break
                ci = cand[0]
                placement[ci][t] = (e, st, ln)
                free.remove(ci)
            if not ok:
                break
        if ok:
            return placement
    return None


def _plan_slots(counts: np.ndarray):
    """Choose per-core slot capacities and expert-piece placement.

    Experts are cut into at most two pieces assigned to a pair of slot
    types.  First the precomputed general cap vectors are tried (exact DP
    feasibility on the actual counts); otherwise a threshold-cut search
    (heavy/light primaries + ranked remainders) provides the fallback.
    Cost model: stage-1 PE time scales with total capacity, stage-2/routing
    with ceil(cap/128) tiles.

    Returns (caps, placement) where placement[core] is a list of
    (expert, tok_start, length) per slot (length may be 0).
    """
    E_ = len(counts)
    order = np.argsort(-counts, kind="stable")
    heavy = [int(e) for e in order[:NCORES]]
    light = [int(e) for e in order[NCORES:]]
    c0 = int(counts[heavy[0]])
    c8 = int(counts[light[0]])

    def plan_cost(caps):
        ct = sum(caps)
        tiles = sum(math.ceil(cp / 128) for cp in caps if cp)
        return 64 * ct + (8 * 512 + 8 * E_) * tiles

    def build(A, C):
        pieces_b = []  # (expert, start, len) remainders
        for e in heavy:
            if counts[e] > A:
                pieces_b.append((e, A, int(counts[e]) - A))
        for e in light:
            if counts[e] > C:
                pieces_b.append((e, C, int(counts[e]) - C))
        if len(pieces_b) > 2 * NCORES:
            return None
        pieces_b.sort(key=lambda p: -p[2])
        bs = pieces_b[:NCORES]
        ds = pieces_b[NCORES:]
        a = min(c0, A)
        b = bs[0][2] if bs else 0
        c = min(c8, C)
        d = ds[0][2] if ds else 0
        caps = (a, b, c, d)
        # piece -> core assignment avoiding same expert twice on one core
        placement = [[None] * 4 for _ in range(NCORES)]
        for i in range(NCORES):
            placement[i][0] = (heavy[i], 0, min(int(counts[heavy[i]]), A))
            placement[i][2] = (light[i], 0, min(int(counts[light[i]]), C))
        for sl, plist in ((1, bs), (3, ds)):
            free = set(range(NCORES))
            for e, st, ln in plist:
                cand = [i for i in free
                        if e != placement[i][0][0] and e != placement[i][2][0]
                        and (placement[i][1] is None or
                             placement[i][1][0] != e)]
                if not cand:
                    return None
                i = cand[0]
                free.discard(i)
                placement[i][sl] = (e, st, ln)
        return caps, placement

    best = None
    lo_a = (c0 + 1) // 2
    lo_c = (c8 + 1) // 2
    cands = [(c0, c8)]
    for A in range(lo_a, c0 + 1, 2):
        for C in range(lo_c, c8 + 1, 2):
            cands.append((A, C))
    for A, C in cands:
        got = build(A, C)
        if got is None:
            continue
        caps, placement = got
        cost = plan_cost(caps)
        if best is None or cost < best[0]:
            best = (cost, caps, placement)
    # precomputed general cap vectors (validated against actual counts)
    experts_desc = [int(e) for e in order]
    counts_desc = [int(counts[e]) for e in experts_desc]
    for caps_c in _CAPS_CANDIDATES:
        if plan_cost(caps_c) >= best[0]:
            continue
        choice = _dp_assign(caps_c, counts_desc)
        if choice is None:
            continue
        pl = _place_from_choice(caps_c, experts_desc, counts, choice)
        if pl is None:
            continue
        best = (plan_cost(caps_c), caps_c, pl)
    _, caps, placement = best
    # drop zero-cap slots; fill empty kept slots with a zero-length piece of
    # some expert not already used by that core (perm needs distinct experts)
    keep = [si for si in range(4) if caps[si] > 0]
    caps_k = tuple(caps[si] for si in keep)
    placement_k = []
    for pl in placement:
        row = []
        used = {p[0] for p in pl if p is not None}
        for si in keep:
            p = pl[si]
            if p is None:
                e_fill = next(e for e in range(E_) if e not in used)
                used.add(e_fill)
                p = (e_fill, 0, 0)
            row.append(p)
        placement_k.append(row)
    return caps_k, placement_k


def _conv_tiles(caps) -> list[tuple[int, int]]:
    """(slot, tail_len) for partial last tiles converted to the
    h-on-partitions stage-2 path; the global-final slot keeps the normal
    path (its tile is the drain-optimized kernel tail)."""
    return [(s, caps[s] % 128) for s in range(len(caps) - 1)
            if caps[s] % 128]


def _chunk_sizes(cap: int, rem_first: bool) -> list[int]:
    """Split cap into <=512-sized chunks; remainder first or last."""
    n_full, rem = divmod(cap, 512)
    sizes = [512] * n_full
    if rem:
        if rem_first:
            sizes = [rem] + sizes
        else:
            sizes = sizes + [rem]
    return sizes


def _build_moe(caps: tuple[int, ...]):
    """One-launch MoE FFN + on-device combine weights.

    Inputs per core (S = len(caps) expert slots):
      w13t  [S, H, 2F] bf16  per-slot hstack(w1[e].T, w3[e].T)
      w2t   [S, F, H]  bf16  per-slot w2[e].T
      xgt   [H, CT]    bf16  gathered tokens (transposed), CT = sum(caps)
      gtp   [H, E]     bf16  gate_w.T, columns permuted so that column s is
                             slot s's expert
      biasp [128, E]   f32   e_score_correction_bias, same permutation,
                             broadcast to 128 partitions
    Output:
      yg    [CT, H]    bf16  combine-weighted expert outputs per gathered token
    """
    S = len(caps)
    CT = sum(caps)
    chunk_lists = [_chunk_sizes(cap, rem_first=False) for cap in caps]
    ntiles_total = sum(math.ceil(tl / 128)
                       for chunks in chunk_lists for tl in chunks)
    # non-final partial tiles are converted to an h-on-partitions stage-2
    # whose matmul cost scales with the real token count instead of a full
    # 512-column sweep; their output goes to yg2 [H, PT] column-major
    conv = _conv_tiles(caps)
    PT = sum(t for _, t in conv)

    nc = bacc.Bacc("TRN2", target_bir_lowering=False, debug=False,
                   num_devices=NCORES)
    w13 = nc.dram_tensor("w13t", [S, H, 2 * F], BF16,
                         kind="ExternalInput").ap()
    w2t = nc.dram_tensor("w2t", [S, F, H], BF16, kind="ExternalInput").ap()
    xgt = nc.dram_tensor("xgt", [H, CT], BF16, kind="ExternalInput").ap()
    gtp = nc.dram_tensor("gtp", [H, E], BF16, kind="ExternalInput").ap()
    biasp = nc.dram_tensor("biasp", [128, E], F32, kind="ExternalInput").ap()
    yg = nc.dram_tensor("yg", [CT, H], BF16, kind="ExternalOutput").ap()
    if conv:
        identf = nc.dram_tensor("identf", [128, 128], F32,
                                kind="ExternalInput").ap()
        yg2 = nc.dram_tensor("yg2", [H, PT], BF16, kind="ExternalOutput").ap()
        yg2_r = yg2.rearrange("(hk p) t -> p hk t", p=128)

    xgt_r = xgt.rearrange("(ko p) t -> p ko t", p=128)
    SIG = mybir.ActivationFunctionType.Sigmoid

    with tile.TileContext(nc) as tc:
        with (
            tc.tile_pool(name="const_p", bufs=1) as const_p,
            tc.tile_pool(name="w13_p", bufs=2) as w13_p,
            tc.tile_pool(name="w2_p", bufs=2) as w2_p,
            tc.tile_pool(name="xg_p", bufs=3) as xg_p,
            tc.tile_pool(name="ht_p", bufs=2) as ht_p,
            tc.tile_pool(name="sg_p", bufs=2) as sg_p,
            tc.tile_pool(name="y_p", bufs=3) as y_p,
            tc.tile_pool(name="work_p", bufs=2) as work_p,
            tc.tile_pool(name="ps", bufs=4, space="PSUM") as ps_pool,
        ):
            gt_sb = const_p.tile([128, KC, E], BF16)
            bias_sb = const_p.tile([128, E], F32)
            w_sb = const_p.tile([128, ntiles_total], F32)
            if conv:
                ident_sb = const_p.tile([128, 128], F32)
                ones_sb = const_p.tile([1, 128], F32)
                nc.gpsimd.memset(ones_sb[:], 1.0)
            conv_map = {}
            pt_off = 0
            for s_c, ttl_c in conv:
                conv_map[s_c] = (ttl_c, pt_off)
                pt_off += ttl_c

            def routing(xg_sb, tl, nt, s, ci, jglob):
                """Combine weight of this slot's expert for one token chunk."""
                ps_r = ps_pool.tile([128, nt, E], F32, tag="psr",
                                    bufs=2, name=f"ps_r_{s}_{ci}")
                # partial last tile leaves rows >= ttl unwritten by the
                # matmuls; zero-fill so batched reads are fully defined
                nc.vector.memset(ps_r[:, :nt, :], 0.0)
                for j in range(nt):
                    tt0 = j * 128
                    ttl = min(128, tl - tt0)
                    for k in range(KC):
                        nc.tensor.matmul(
                            ps_r[:ttl, j, :],
                            lhsT=xg_sb[k][:, tt0:tt0 + ttl],
                            rhs=gt_sb[:, k, :],
                            start=(k == 0), stop=(k == KC - 1))
                # scores = sigmoid(logits) = 0.5 + 0.5*tanh(logits/2); Tanh
                # shares the activation table with Silu (no table reloads)
                th = work_p.tile([128, nt, E], F32, tag="th",
                                 name=f"th_{s}_{ci}", padded_shape=[128, 4, E])
                nc.scalar.activation(
                    th[:, :nt, :], ps_r[:, :nt, :],
                    mybir.ActivationFunctionType.Tanh, scale=0.5)
                sc = work_p.tile([128, nt, E], F32, tag="sc",
                                 name=f"sc_{s}_{ci}", padded_shape=[128, 4, E])
                nc.vector.tensor_scalar(
                    sc[:, :nt, :], th[:, :nt, :], 0.5, 0.5,
                    op0=mybir.AluOpType.mult, op1=mybir.AluOpType.add)
                biased = work_p.tile([128, nt, E], F32, tag="biased",
                                     name=f"biased_{s}_{ci}",
                                     padded_shape=[128, 4, E])
                nc.vector.tensor_tensor(
                    biased[:, :nt, :], sc[:, :nt, :],
                    bias_sb[:, None, :].to_broadcast([128, nt, E]),
                    mybir.AluOpType.add)
                m8 = work_p.tile([128, nt, 8], F32, tag="m8",
                                 name=f"m8_{s}_{ci}", padded_shape=[128, 4, 8])
                sel = work_p.tile([128, nt, E], F32, tag="sel",
                                  name=f"sel_{s}_{ci}",
                                  padded_shape=[128, 4, E])
                for j in range(nt):
                    nc.vector.max(m8[:, j, :], biased[:, j, :])
                for j in range(nt):
                    nc.vector.tensor_scalar(
                        sel[:, j, :], biased[:, j, :],
                        m8[:, j, TOPK - 1:TOPK], None,
                        op0=mybir.AluOpType.is_ge)
                picked = work_p.tile([128, nt, E], F32, tag="picked",
                                     name=f"picked_{s}_{ci}",
                                     padded_shape=[128, 4, E])
                nc.vector.tensor_mul(
                    picked[:, :nt, :], sel[:, :nt, :], sc[:, :nt, :])
                denom = work_p.tile([128, nt], F32, tag="denom",
                                    name=f"denom_{s}_{ci}",
                                    padded_shape=[128, 4])
                nc.vector.reduce_sum(
                    denom[:, :nt], picked[:, :nt, :], axis=mybir.AxisListType.X)
                recip = work_p.tile([128, nt], F32, tag="recip",
                                    name=f"recip_{s}_{ci}",
                                    padded_shape=[128, 4])
                nc.vector.reciprocal(recip[:, :nt], denom[:, :nt])
                # slot expert score is column s (host permutation)
                nc.vector.tensor_mul(
                    w_sb[:, jglob:jglob + nt], sc[:, :nt, s], recip[:, :nt])

            def evac_stage1(ps_g, ps_u, ht_sb, fi, tl):
                """ht[:, fi, :tl] = silu(g) * u."""
                sg = sg_p.tile([128, 512], F32, tag="sgm", name=f"sg_{fi}")
                nc.scalar.activation(sg[:, :tl], ps_g[:, :tl],
                                     mybir.ActivationFunctionType.Silu)
                nc.vector.tensor_mul(
                    ht_sb[:, fi, :tl], sg[:, :tl], ps_u[:, :tl])

            def emit_stage2(p, final):
                """Stage 2 for one chunk: y[t,h] = w[t]*sum_f hT[f,t]*w2T[f,h].

                Emitted one chunk late (software pipeline) so the PE never
                waits on the chunk's own ht evacuation chain.
                """
                tl, nt, t0 = p["tl"], p["nt"], p["t0"]
                ht_sb, w2_sb, jg = p["ht_sb"], p["w2_sb"], p["jglob"]
                for j in range(nt):
                    if j == nt - 1 and p.get("conv"):
                        # h-on-partitions tail tile: 8 h-chunks x 4 kf
                        # matmuls of ap=token-count; output column-major
                        ttl_c, po, htw = p["conv"]
                        y2_sb = y_p.tile([128, KC, 128], BF16, tag="y2",
                                         name=f"y2_sb_{po}")
                        for hk in range(KC):
                            ps_h = ps_pool.tile([128, 512], F32, tag="psy",
                                                bufs=3,
                                                name=f"ps_h_{po}_{hk}")
                            for kf in range(FC):
                                nc.tensor.matmul(
                                    ps_h[:, :ttl_c],
                                    lhsT=w2_sb[:, kf,
                                               hk * 128:(hk + 1) * 128],
                                    rhs=htw[:, kf, :ttl_c],
                                    start=(kf == 0), stop=(kf == FC - 1))
                            if hk % 2 == 0:
                                nc.scalar.activation(
                                    y2_sb[:, hk, :ttl_c], ps_h[:, :ttl_c],
                                    mybir.ActivationFunctionType.Copy)
                            else:
                                nc.vector.tensor_copy(
                                    y2_sb[:, hk, :ttl_c], ps_h[:, :ttl_c])
                        nc.sync.dma_start(
                            yg2_r[:, :, po:po + ttl_c],
                            y2_sb[:, :, :ttl_c])
                        continue
                    tt0 = j * 128
                    ttl = min(128, tl - tt0)
                    wj = w_sb[:ttl, jg + j:jg + j + 1]
                    y_sb = y_p.tile([128, H], BF16, tag="y", name=f"y_sb_{jg+j}")
                    ps_ys = []
                    for hh in range(2):
                        ps_y = ps_pool.tile([128, 512], F32, tag="psy",
                                            bufs=3, name=f"ps_y_{jg+j}_{hh}")
                        ps_ys.append(ps_y)
                        for kf in range(FC):
                            nc.tensor.matmul(
                                ps_y[:ttl],
                                lhsT=ht_sb[:, kf, tt0:tt0 + ttl],
                                rhs=w2_sb[:, kf, hh * 512:(hh + 1) * 512],
                                start=(kf == 0), stop=(kf == FC - 1))
                        if hh == 0 or not (final and j == nt - 1):
                            nc.vector.tensor_scalar(
                                y_sb[:ttl, hh * 512:(hh + 1) * 512],
                                ps_y[:ttl], wj, None,
                                op0=mybir.AluOpType.mult)
                    if final and j == nt - 1:
                        # final tile: drain the second half as two quarter
                        # pieces on parallel engines/queues to shorten the
                        # end-of-kernel DMA latency chain
                        rows = slice(t0 + tt0, t0 + tt0 + ttl)
                        nc.sync.dma_start(
                            yg[rows, 0:512], y_sb[:ttl, 0:512])
                        nc.scalar.activation(
                            y_sb[:ttl, 512:768], ps_ys[1][:ttl, 0:256],
                            mybir.ActivationFunctionType.Copy,
                            scale=wj)
                        nc.scalar.dma_start(
                            yg[rows, 512:768], y_sb[:ttl, 512:768])
                        ybq = y_p.tile([128, 256], BF16, tag="ybq",
                                       bufs=1, name="ybq_last")
                        nc.vector.tensor_scalar(
                            ybq[:ttl, :], ps_ys[1][:ttl, 256:512],
                            wj, None, op0=mybir.AluOpType.mult)
                        nc.sync.dma_start(
                            yg[rows, 768:1024], ybq[:ttl, :])
                    else:
                        nc.sync.dma_start(
                            yg[t0 + tt0:t0 + tt0 + ttl, :], y_sb[:ttl, :])

            jglob = 0
            pending = None
            for s in range(S):
                cap = caps[s]
                off = sum(caps[:s])
                chunks = chunk_lists[s]

                # k=0 weights split into g/u halves so the first matmul's DMA
                # dependency is small; k>=1 combined to halve the issue count
                w13g0 = w13_p.tile([128, F], BF16, tag="w13g0",
                                   name=f"w13g0_{s}")
                w13u0 = w13_p.tile([128, F], BF16, tag="w13u0",
                                   name=f"w13u0_{s}")
                nc.sync.dma_start(w13g0[:], w13[s, 0:128, 0:F])
                nc.sync.dma_start(w13u0[:], w13[s, 0:128, F:2 * F])
                w13k = [None] + [w13_p.tile([128, 2 * F], BF16,
                                            tag=f"w13_{k}",
                                            name=f"w13_sb_{s}_{k}")
                                 for k in range(1, KC)]
                for k in range(1, KC):
                    nc.sync.dma_start(
                        w13k[k][:], w13[s, k * 128:(k + 1) * 128, :])

                def gv(k, fi):
                    if k == 0:
                        return w13g0[:, fi * 128:(fi + 1) * 128]
                    return w13k[k][:, fi * 128:(fi + 1) * 128]

                def uv(k, fi):
                    if k == 0:
                        return w13u0[:, fi * 128:(fi + 1) * 128]
                    return w13k[k][:, F + fi * 128:F + (fi + 1) * 128]

                tch0 = 0
                for ci, tl in enumerate(chunks):
                    t0 = off + tch0
                    tch0 += tl
                    nt = math.ceil(tl / 128)
                    ramp = (s == 0 and ci == 0)

                    xg_big = xg_p.tile([128, KC, 512], BF16, tag="xg",
                                       name=f"xg_sb_{s}_{ci}")
                    xg_sb = [xg_big[:, k, :] for k in range(KC)]
                    if ramp:
                        # per-k DMAs so the PE can consume k-chunks as they
                        # stream in during the cold start
                        for k in range(KC):
                            nc.scalar.dma_start(
                                xg_big[:, k, :tl], xgt_r[:, k, t0:t0 + tl])
                    else:
                        nc.sync.dma_start(
                            xg_big[:, :, :tl], xgt_r[:, :, t0:t0 + tl])
                    if ramp:
                        # routing consts + slot-0 w2 AFTER the ramp-critical
                        # xg tiles (a big early w2 transfer would stall the
                        # first matmuls behind it on the shared DMA engines)
                        nc.scalar.dma_start(
                            gt_sb[:],
                            gtp.rearrange("(ko p) e -> p ko e", p=128))
                        nc.scalar.dma_start(bias_sb[:], biasp[:])
                        if conv:
                            nc.scalar.dma_start(ident_sb[:], identf[:])
                    if ci == 0:
                        w2_sb = w2_p.tile([128, FC, H], BF16, tag="w2",
                                          name=f"w2_sb_{s}")
                        nc.scalar.dma_start(
                            w2_sb[:],
                            w2t[s].rearrange("(ko p) h -> p ko h", p=128))

                    ht_sb = ht_p.tile([128, FC, 512], BF16, tag="ht")

                    if not ramp:
                        # routing first: its ACT+DVE chain then completes
                        # during stage 1, well before stage 2 consumes w_sb
                        routing(xg_sb, tl, nt, s, ci, jglob)

                    # ---- stage 1: hT[f,t] = silu(x@w1.T).T * (x@w3.T).T ----
                    if ramp:
                        # k OUTER across all fi: the PE consumes each
                        # weight/activation k-chunk as it streams in.
                        # 8 live PSUM tiles across the three tags.
                        tags = ["ps1", "ps1", "ps1", "psy",
                                "psy", "psy", "psr", "psr"]
                        ps8 = [ps_pool.tile([128, 512], F32, tag=tags[i],
                                            bufs=(2 if tags[i] == "psr"
                                                  else 3),
                                            name=f"ps_ramp_{i}")
                               for i in range(8)]
                        ps_gs = ps8[0::2]
                        ps_us = ps8[1::2]
                        for k in range(KC):
                            for fi in range(FC):
                                nc.tensor.matmul(
                                    ps_gs[fi][:, :tl], lhsT=gv(k, fi),
                                    rhs=xg_sb[k][:, :tl],
                                    start=(k == 0), stop=(k == KC - 1))
                                nc.tensor.matmul(
                                    ps_us[fi][:, :tl], lhsT=uv(k, fi),
                                    rhs=xg_sb[k][:, :tl],
                                    start=(k == 0), stop=(k == KC - 1))
                        for fi in range(FC):
                            evac_stage1(ps_gs[fi], ps_us[fi], ht_sb, fi, tl)
                        # ramp routing last (needs every xg k-chunk anyway)
                        routing(xg_sb, tl, nt, s, ci, jglob)
                    else:
                        # fi sequential, k inner: only 2 PSUM tiles live
                        for fi in range(FC):
                            ps_g = ps_pool.tile([128, 512], F32, tag="ps1",
                                                bufs=3,
                                                name=f"ps_g_{s}_{ci}_{fi}")
                            ps_u = ps_pool.tile([128, 512], F32, tag="ps1",
                                                bufs=3,
                                                name=f"ps_u_{s}_{ci}_{fi}")
                            for k in range(KC):
                                nc.tensor.matmul(
                                    ps_g[:, :tl], lhsT=gv(k, fi),
                                    rhs=xg_sb[k][:, :tl],
                                    start=(k == 0), stop=(k == KC - 1))
                                nc.tensor.matmul(
                                    ps_u[:, :tl], lhsT=uv(k, fi),
                                    rhs=xg_sb[k][:, :tl],
                                    start=(k == 0), stop=(k == KC - 1))
                            evac_stage1(ps_g, ps_u, ht_sb, fi, tl)

                    # ---- stage 2 of the PREVIOUS chunk (pipelined) ----
                    if pending is not None:
                        emit_stage2(pending, final=False)
                    pending = {"tl": tl, "nt": nt, "t0": t0, "ht_sb": ht_sb,
                               "w2_sb": w2_sb, "jglob": jglob}
                    if ci == len(chunks) - 1 and s in conv_map:
                        # broadcast the tail tile's combine weights along the
                        # free dim (PE transpose + rank-1 matmul), pre-scale
                        # ht; its stage 2 then runs h-on-partitions with
                        # matmul cost proportional to the real token count
                        ttl_c, po = conv_map[s]
                        tt0p = (nt - 1) * 128
                        jcol = jglob + nt - 1
                        wrow_ps = ps_pool.tile([128, 512], F32, tag="psr",
                                               bufs=2, name=f"wrow_ps_{s}")
                        nc.tensor.transpose(
                            wrow_ps[0:1, 0:128], w_sb[:, jcol:jcol + 1],
                            ident_sb[:])
                        wrow_sb = work_p.tile([1, 128], F32, tag="wrow",
                                              name=f"wrow_sb_{s}")
                        nc.vector.tensor_copy(
                            wrow_sb[0:1, :], wrow_ps[0:1, 0:128])
                        wb_ps = ps_pool.tile([128, 512], F32, tag="psr",
                                             bufs=2, name=f"wb_ps_{s}")
                        nc.tensor.matmul(
                            wb_ps[:, :ttl_c], lhsT=ones_sb[0:1, :],
                            rhs=wrow_sb[0:1, :ttl_c], start=True, stop=True)
                        wbs = work_p.tile([128, 128], F32, tag="wbs",
                                          name=f"wbs_{s}")
                        nc.vector.tensor_copy(
                            wbs[:, :ttl_c], wb_ps[:, :ttl_c])
                        htw = ht_p.tile([128, FC, 128], BF16, tag="htw",
                                        name=f"htw_{s}")
                        nc.vector.tensor_tensor(
                            htw[:, :, :ttl_c],
                            ht_sb[:, :, tt0p:tt0p + ttl_c],
                            wbs[:, None, :ttl_c].to_broadcast(
                                [128, FC, ttl_c]),
                            mybir.AluOpType.mult)
                        pending["conv"] = (ttl_c, po, htw)
                    jglob += nt

            emit_stage2(pending, final=True)

    nc.compile()
    return nc


def _moe_nc(caps):
    key = ("moe", caps)
    if key not in _nc_cache:
        _nc_cache[key] = _build_moe(caps)
    return _nc_cache[key]


def kernel(hidden_states, gate_w, bias, w1, w3, w2):
    x = np.ascontiguousarray(np.asarray(hidden_states, dtype=np.float32))
    gate_w = np.asarray(gate_w, dtype=np.float32)
    bias = np.asarray(bias, dtype=np.float32)
    w1 = np.asarray(w1, dtype=np.float32)
    w3 = np.asarray(w3, dtype=np.float32)
    w2 = np.asarray(w2, dtype=np.float32)

    # ---- Host dispatch: fp32 routing decides token->expert placement ----
    logits = x @ gate_w.T                                # [T, E]
    scores = 1.0 / (1.0 + np.exp(-logits))
    biased = scores + bias[None, :]
    topi = np.argpartition(-biased, TOPK - 1, axis=1)[:, :TOPK]  # [T, K] sets
    sel = np.zeros((T, E), dtype=bool)
    sel[np.arange(T)[:, None], topi] = True
    idx_per_e = [np.nonzero(sel[:, e])[0] for e in range(E)]
    counts = np.array([len(ix) for ix in idx_per_e])
    caps, placement = _plan_slots(counts)
    S = len(caps)
    offs = [sum(caps[:si]) for si in range(S)]
    global LAST_CAPS
    LAST_CAPS = caps
    CT = sum(caps)

    xT = np.ascontiguousarray(x.T)                       # [H, T]
    xT16 = xT.astype(ml_dtypes.bfloat16)
    gT16 = np.ascontiguousarray(gate_w.T).astype(ml_dtypes.bfloat16)

    in_maps = []
    for c in range(NCORES):
        slot_experts = [p[0] for p in placement[c]]
        idx_pad = np.zeros(CT, dtype=np.int64)
        for si, (e, st, ln) in enumerate(placement[c]):
            if ln:
                idx_pad[offs[si]:offs[si] + ln] = idx_per_e[e][st:st + ln]
        xgt = np.ascontiguousarray(xT16[:, idx_pad])     # [H, CT] bf16
        w13t = np.stack([
            np.ascontiguousarray(
                np.concatenate([w1[e].T, w3[e].T], axis=1))
            for e in slot_experts]).astype(ml_dtypes.bfloat16)  # [S, H, 2F]
        w2t = np.stack(
            [np.ascontiguousarray(w2[e].T) for e in slot_experts]
        ).astype(ml_dtypes.bfloat16)
        perm = slot_experts + [e for e in range(E) if e not in slot_experts]
        gtp = np.ascontiguousarray(gT16[:, perm])        # [H, E] bf16
        biasp = np.ascontiguousarray(
            np.broadcast_to(np.asarray(bias)[perm][None, :],
                            (128, E))).astype(np.float32)
        im = {"w13t": w13t, "w2t": w2t, "xgt": xgt, "gtp": gtp,
              "biasp": biasp}
        if _conv_tiles(caps):
            im["identf"] = np.eye(128, dtype=np.float32)
        in_maps.append(im)

    # ---- Single SPMD launch: routing weights + expert FFN ----
    ncB = _moe_nc(caps)
    res = run_bass_kernel_spmd(ncB, in_maps, core_ids=list(range(NCORES)))

    # ---- Host combine: scatter-add (tail-tile rows come from yg2) ----
    conv_map = {}
    po = 0
    for s_c, ttl_c in _conv_tiles(caps):
        conv_map[s_c] = (ttl_c, po)
        po += ttl_c
    out = np.zeros((T, H), dtype=np.float32)
    for c in range(NCORES):
        yg_c = res.results[c]["yg"]
        yg2_c = res.results[c].get("yg2")
        for si, (e, st, ln) in enumerate(placement[c]):
            if not ln:
                continue
            ix = idx_per_e[e][st:st + ln]
            t0p = (math.ceil(caps[si] / 128) - 1) * 128
            if si in conv_map and ln > t0p:
                ttl_c, po = conv_map[si]
                seg = np.concatenate([
                    yg_c[offs[si]:offs[si] + t0p].astype(np.float32),
                    yg2_c[:, po:po + (ln - t0p)].T.astype(np.float32)],
                    axis=0)
            else:
                seg = yg_c[offs[si]:offs[si] + ln].astype(np.float32)
            out[ix] += seg
    return out


# revision 42
# speedup vs baseline: 1.0508x; 1.0105x over previous
"""MiniMax-M2 MoE kernel for 8 Trainium2 NeuronCores.

Single-launch expert-parallel design:
  Host (data movement / dispatch only): fp32 routing decides WHICH tokens go
    to WHICH expert (indices only); a planner cuts each expert's token list
    into at most two pieces and packs them into up to 4 static expert slots
    per core (capacities chosen to minimize modeled PE time, ~7% over the
    perfect-balance floor); tokens are gathered per slot and weights
    pre-transposed/cast to bf16.
  Device (all output-value arithmetic, one SPMD launch):
    - per slot, recompute router scores for the slot's gathered tokens
      (logits -> sigmoid -> top-4 threshold on bias-corrected scores ->
      renormalized combine weight of the slot's own expert; the host permutes
      the gate matrix per core so slot s's expert is always column s),
    - SwiGLU FFN (bf16 matmuls) and combine-weight scaling.  silu(g) is
      computed as g * sigmoid(g) so the Activation engine only ever needs the
      sigmoid table (one LoadActFuncSet instead of thrashing Silu<->Sigmoid).
    - stage 2 runs one chunk behind stage 1 (software pipeline) so the PE
      never idles on the ht evacuation chain.
  Host: scatter-add per-slot outputs into [T, H].
"""

import math

import ml_dtypes
import numpy as np

import concourse.bass as bass
import concourse.tile as tile
from concourse import bacc, mybir
from concourse.bass_utils import run_bass_kernel_spmd

T, H, F, E, TOPK = 4096, 1024, 512, 16, 4
NCORES = 8
KC = H // 128   # contraction chunks (hidden dim)
FC = F // 128   # stage-2 contraction chunks
F32 = mybir.dt.float32
BF16 = mybir.dt.bfloat16

_nc_cache: dict = {}
LAST_CAPS = (832, 492, 512, 354)  # caps used by the most recent kernel() call


# Good general cap vectors found by offline search on the canonical routing
# distribution; each is validated against the ACTUAL counts at runtime (DP
# feasibility + placement construction) before use.
_CAPS_CANDIDATES = [(684, 604, 460, 350)]


def _dp_assign(caps, counts_desc):
    """Assign each expert (counts desc) a pair of slot types (i<=j) such
    that caps[i]+caps[j] >= count and each type is used at most 8 times.
    Returns the choice list or None."""
    pairs = [(i, j) for i in range(len(caps)) for j in range(i, len(caps))]
    capsum = {p: caps[p[0]] + caps[p[1]] for p in pairs}
    opts = []
    for c in counts_desc:
        o = [p for p in pairs if capsum[p] >= c]
        if not o:
            return None
        opts.append(o)
    n = len(counts_desc)
    seen = set()
    choice = [None] * n

    def dfs(k, rem):
        if k == n:
            return True
        key = (k, rem)
        if key in seen:
            return False
        for (i, j) in opts[k]:
            r2 = list(rem)
            r2[i] -= 1
            r2[j] -= 1
            if r2[i] >= 0 and r2[j] >= 0:
                choice[k] = (i, j)
                if dfs(k + 1, tuple(r2)):
                    return True
        seen.add(key)
        return False

    if not dfs(0, (NCORES,) * len(caps)):
        return None
    return choice


def _place_from_choice(caps, experts_desc, counts, choice):
    """Build placement[core][slot] = (expert, tok_start, len) from a
    type-pair assignment; both pieces of one expert land on distinct cores.
    Returns placement or None."""
    S = len(caps)
    pieces_per_type = [[] for _ in range(S)]
    for k, e in enumerate(experts_desc):
        i, j = choice[k]
        c = int(counts[e])
        pi = min(caps[i], c)
        pj = c - pi
        pieces_per_type[i].append((e, 0, pi))
        pieces_per_type[j].append((e, pi, pj))
    for rot in range(NCORES):
        placement = [[None] * S for _ in range(NCORES)]
        ok = True
        for t in range(S):
            free = list(range(NCORES))
            free = free[rot:] + free[:rot]
            for (e, st, ln) in sorted(pieces_per_type[t],
                                      key=lambda p: -p[2]):
                cand = [ci for ci in free
                        if e not in {p[0] for p in placement[ci] if p}]
                if not cand:
                    ok = False
                    break
                ci = cand[0]
                placement[ci][t] = (e, st, ln)
                free.remove(ci)
            if not ok:
                break
        if ok:
            return placement
    return None


def _plan_slots(counts: np.ndarray):
    """Choose per-core slot capacities and expert-piece placement.

    Experts are cut into at most two pieces assigned to a pair of slot
    types.  First the precomputed general cap vectors are tried (exact DP
    feasibility on the actual counts); otherwise a threshold-cut search
    (heavy/light primaries + ranked remainders) provides the fallback.
    Cost model: stage-1 PE time scales with total capacity, stage-2/routing
    with ceil(cap/128) tiles.

    Returns (caps, placement) where placement[core] is a list of
    (expert, tok_start, length) per slot (length may be 0).
    """
    E_ = len(counts)
    order = np.argsort(-counts, kind="stable")
    heavy = [int(e) for e in order[:NCORES]]
    light = [int(e) for e in order[NCORES:]]
    c0 = int(counts[heavy[0]])
    c8 = int(counts[light[0]])

    def plan_cost(caps):
        ct = sum(caps)
        tiles = sum(math.ceil(cp / 128) for cp in caps if cp)
        return 64 * ct + (8 * 512 + 8 * E_) * tiles

    def build(A, C):
        pieces_b = []  # (expert, start, len) remainders
        for e in heavy:
            if counts[e] > A:
                pieces_b.append((e, A, int(counts[e]) - A))
        for e in light:
            if counts[e] > C:
                pieces_b.append((e, C, int(counts[e]) - C))
        if len(pieces_b) > 2 * NCORES:
            return None
        pieces_b.sort(key=lambda p: -p[2])
        bs = pieces_b[:NCORES]
        ds = pieces_b[NCORES:]
        a = min(c0, A)
        b = bs[0][2] if bs else 0
        c = min(c8, C)
        d = ds[0][2] if ds else 0
        caps = (a, b, c, d)
        # piece -> core assignment avoiding same expert twice on one core
        placement = [[None] * 4 for _ in range(NCORES)]
        for i in range(NCORES):
            placement[i][0] = (heavy[i], 0, min(int(counts[heavy[i]]), A))
            placement[i][2] = (light[i], 0, min(int(counts[light[i]]), C))
        for sl, plist in ((1, bs), (3, ds)):
            free = set(range(NCORES))
            for e, st, ln in plist:
                cand = [i for i in free
                        if e != placement[i][0][0] and e != placement[i][2][0]
                        and (placement[i][1] is None or
                             placement[i][1][0] != e)]
                if not cand:
                    return None
                i = cand[0]
                free.discard(i)
                placement[i][sl] = (e, st, ln)
        return caps, placement

    best = None
    lo_a = (c0 + 1) // 2
    lo_c = (c8 + 1) // 2
    cands = [(c0, c8)]
    for A in range(lo_a, c0 + 1, 2):
        for C in range(lo_c, c8 + 1, 2):
            cands.append((A, C))
    for A, C in cands:
        got = build(A, C)
        if got is None:
            continue
        caps, placement = got
        cost = plan_cost(caps)
        if best is None or cost < best[0]:
            best = (cost, caps, placement)
    # precomputed general cap vectors (validated against actual counts)
    experts_desc = [int(e) for e in order]
    counts_desc = [int(counts[e]) for e in experts_desc]
    for caps_c in _CAPS_CANDIDATES:
        if plan_cost(caps_c) >= best[0]:
            continue
        choice = _dp_assign(caps_c, counts_desc)
        if choice is None:
            continue
        pl = _place_from_choice(caps_c, experts_desc, counts, choice)
        if pl is None:
            continue
        best = (plan_cost(caps_c), caps_c, pl)
    _, caps, placement = best
    # drop zero-cap slots; fill empty kept slots with a zero-length piece of
    # some expert not already used by that core (perm needs distinct experts)
    keep = [si for si in range(4) if caps[si] > 0]
    caps_k = tuple(caps[si] for si in keep)
    placement_k = []
    for pl in placement:
        row = []
        used = {p[0] for p in pl if p is not None}
        for si in keep:
            p = pl[si]
            if p is None:
                e_fill = next(e for e in range(E_) if e not in used)
                used.add(e_fill)
                p = (e_fill, 0, 0)
            row.append(p)
        placement_k.append(row)
    return caps_k, placement_k


def _conv_tiles(caps) -> list[tuple[int, int]]:
    """(slot, tail_len) for partial last tiles converted to the
    h-on-partitions stage-2 path; the global-final slot keeps the normal
    path (its tile is the drain-optimized kernel tail)."""
    return [(s, caps[s] % 128) for s in range(len(caps) - 1)
            if caps[s] % 128]


def _chunk_sizes(cap: int, rem_first: bool) -> list[int]:
    """Split cap into <=512-sized chunks; remainder first or last."""
    n_full, rem = divmod(cap, 512)
    sizes = [512] * n_full
    if rem:
        if rem_first:
            sizes = [rem] + sizes
        else:
            sizes = sizes + [rem]
    return sizes


def _build_moe(caps: tuple[int, ...]):
    """One-launch MoE FFN + on-device combine weights.

    Inputs per core (S = len(caps) expert slots):
      w13t  [S, H, 2F] bf16  per-slot hstack(w1[e].T, w3[e].T)
      w2t   [S, F, H]  bf16  per-slot w2[e].T
      xgt   [H, CT]    bf16  gathered tokens (transposed), CT = sum(caps)
      gtp   [H, E]     bf16  gate_w.T, columns permuted so that column s is
                             slot s's expert
      biasp [128, E]   f32   e_score_correction_bias, same permutation,
                             broadcast to 128 partitions
    Output:
      yg    [CT, H]    bf16  combine-weighted expert outputs per gathered token
    """
    S = len(caps)
    CT = sum(caps)
    chunk_lists = [_chunk_sizes(cap, rem_first=False) for cap in caps]
    ntiles_total = sum(math.ceil(tl / 128)
                       for chunks in chunk_lists for tl in chunks)
    # non-final partial tiles are converted to an h-on-partitions stage-2
    # whose matmul cost scales with the real token count instead of a full
    # 512-column sweep; their output goes to yg2 [H, PT] column-major
    conv = _conv_tiles(caps)
    PT = sum(t for _, t in conv)

    nc = bacc.Bacc("TRN2", target_bir_lowering=False, debug=False,
                   num_devices=NCORES)
    w13 = nc.dram_tensor("w13t", [S, H, 2 * F], BF16,
                         kind="ExternalInput").ap()
    w2t = nc.dram_tensor("w2t", [S, F, H], BF16, kind="ExternalInput").ap()
    xgt = nc.dram_tensor("xgt", [H, CT], BF16, kind="ExternalInput").ap()
    gtp = nc.dram_tensor("gtp", [H, E], BF16, kind="ExternalInput").ap()
    biasp = nc.dram_tensor("biasp", [128, E], F32, kind="ExternalInput").ap()
    yg = nc.dram_tensor("yg", [CT, H], BF16, kind="ExternalOutput").ap()
    if conv:
        identf = nc.dram_tensor("identf", [128, 128], F32,
                                kind="ExternalInput").ap()
        yg2 = nc.dram_tensor("yg2", [H, PT], BF16, kind="ExternalOutput").ap()
        yg2_r = yg2.rearrange("(hk p) t -> p hk t", p=128)

    xgt_r = xgt.rearrange("(ko p) t -> p ko t", p=128)
    SIG = mybir.ActivationFunctionType.Sigmoid

    with tile.TileContext(nc) as tc:
        with (
            tc.tile_pool(name="const_p", bufs=1) as const_p,
            tc.tile_pool(name="w13_p", bufs=2) as w13_p,
            tc.tile_pool(name="w2_p", bufs=2) as w2_p,
            tc.tile_pool(name="xg_p", bufs=3) as xg_p,
            tc.tile_pool(name="ht_p", bufs=2) as ht_p,
            tc.tile_pool(name="sg_p", bufs=2) as sg_p,
            tc.tile_pool(name="y_p", bufs=3) as y_p,
            tc.tile_pool(name="work_p", bufs=2) as work_p,
            tc.tile_pool(name="ps", bufs=4, space="PSUM") as ps_pool,
        ):
            gt_sb = const_p.tile([128, KC, E], BF16)
            bias_sb = const_p.tile([128, E], F32)
            w_sb = const_p.tile([128, ntiles_total], F32)
            if conv:
                ident_sb = const_p.tile([128, 128], F32)
                ones_sb = const_p.tile([1, 128], F32)
                nc.gpsimd.memset(ones_sb[:], 1.0)
            conv_map = {}
            pt_off = 0
            for s_c, ttl_c in conv:
                conv_map[s_c] = (ttl_c, pt_off)
                pt_off += ttl_c

            def routing(xg_sb, tl, nt, s, ci, jglob):
                """Combine weight of this slot's expert for one token chunk."""
                ps_r = ps_pool.tile([128, nt, E], F32, tag="psr",
                                    bufs=2, name=f"ps_r_{s}_{ci}")
                # partial last tile leaves rows >= ttl unwritten by the
                # matmuls; zero-fill so batched reads are fully defined
                nc.vector.memset(ps_r[:, :nt, :], 0.0)
                for j in range(nt):
                    tt0 = j * 128
                    ttl = min(128, tl - tt0)
                    for k in range(KC):
                        nc.tensor.matmul(
                            ps_r[:ttl, j, :],
                            lhsT=xg_sb[k][:, tt0:tt0 + ttl],
                            rhs=gt_sb[:, k, :],
                            start=(k == 0), stop=(k == KC - 1))
                # scores = sigmoid(logits) = 0.5 + 0.5*tanh(logits/2); Tanh
                # shares the activation table with Silu (no table reloads)
                th = work_p.tile([128, nt, E], F32, tag="th",
                                 name=f"th_{s}_{ci}", padded_shape=[128, 4, E])
                nc.scalar.activation(
                    th[:, :nt, :], ps_r[:, :nt, :],
                    mybir.ActivationFunctionType.Tanh, scale=0.5)
                sc = work_p.tile([128, nt, E], F32, tag="sc",
                                 name=f"sc_{s}_{ci}", padded_shape=[128, 4, E])
                nc.vector.tensor_scalar(
                    sc[:, :nt, :], th[:, :nt, :], 0.5, 0.5,
                    op0=mybir.AluOpType.mult, op1=mybir.AluOpType.add)
                biased = work_p.tile([128, nt, E], F32, tag="biased",
                                     name=f"biased_{s}_{ci}",
                                     padded_shape=[128, 4, E])
                nc.vector.tensor_tensor(
                    biased[:, :nt, :], sc[:, :nt, :],
                    bias_sb[:, None, :].to_broadcast([128, nt, E]),
                    mybir.AluOpType.add)
                m8 = work_p.tile([128, nt, 8], F32, tag="m8",
                                 name=f"m8_{s}_{ci}", padded_shape=[128, 4, 8])
                sel = work_p.tile([128, nt, E], F32, tag="sel",
                                  name=f"sel_{s}_{ci}",
                                  padded_shape=[128, 4, E])
                for j in range(nt):
                    nc.vector.max(m8[:, j, :], biased[:, j, :])
                for j in range(nt):
                    nc.vector.tensor_scalar(
                        sel[:, j, :], biased[:, j, :],
                        m8[:, j, TOPK - 1:TOPK], None,
                        op0=mybir.AluOpType.is_ge)
                picked = work_p.tile([128, nt, E], F32, tag="picked",
                                     name=f"picked_{s}_{ci}",
                                     padded_shape=[128, 4, E])
                nc.vector.tensor_mul(
                    picked[:, :nt, :], sel[:, :nt, :], sc[:, :nt, :])
                denom = work_p.tile([128, nt], F32, tag="denom",
                                    name=f"denom_{s}_{ci}",
                                    padded_shape=[128, 4])
                nc.vector.reduce_sum(
                    denom[:, :nt], picked[:, :nt, :], axis=mybir.AxisListType.X)
                recip = work_p.tile([128, nt], F32, tag="recip",
                                    name=f"recip_{s}_{ci}",
                                    padded_shape=[128, 4])
                nc.vector.reciprocal(recip[:, :nt], denom[:, :nt])
                # slot expert score is column s (host permutation)
                nc.vector.tensor_mul(
                    w_sb[:, jglob:jglob + nt], sc[:, :nt, s], recip[:, :nt])

            def evac_stage1(ps_g, ps_u, ht_sb, fi, tl):
                """ht[:, fi, :tl] = silu(g) * u."""
                sg = sg_p.tile([128, 512], F32, tag="sgm", name=f"sg_{fi}")
                nc.scalar.activation(sg[:, :tl], ps_g[:, :tl],
                                     mybir.ActivationFunctionType.Silu)
                nc.vector.tensor_mul(
                    ht_sb[:, fi, :tl], sg[:, :tl], ps_u[:, :tl])

            def emit_stage2(p, final):
                """Stage 2 for one chunk: y[t,h] = w[t]*sum_f hT[f,t]*w2T[f,h].

                Emitted one chunk late (software pipeline) so the PE never
                waits on the chunk's own ht evacuation chain.
                """
                tl, nt, t0 = p["tl"], p["nt"], p["t0"]
                ht_sb, w2_sb, jg = p["ht_sb"], p["w2_sb"], p["jglob"]
                for j in range(nt):
                    if j == nt - 1 and p.get("conv"):
                        # h-on-partitions tail tile: 8 h-chunks x 4 kf
                        # matmuls of ap=token-count; 4 h-chunks share one
                        # PSUM bank so evacuation is 2 batched copies
                        ttl_c, po, htw = p["conv"]
                        y2_sb = y_p.tile([128, KC, 128], BF16, tag="y2",
                                         name=f"y2_sb_{po}")
                        for g in range(2):
                            ps_h4 = ps_pool.tile([128, 4, 128], F32,
                                                 tag="psy", bufs=3,
                                                 name=f"ps_h4_{po}_{g}")
                            for hkk in range(4):
                                hk = g * 4 + hkk
                                for kf in range(FC):
                                    nc.tensor.matmul(
                                        ps_h4[:, hkk, :ttl_c],
                                        lhsT=w2_sb[:, kf,
                                                   hk * 128:(hk + 1) * 128],
                                        rhs=htw[:, kf, :ttl_c],
                                        start=(kf == 0), stop=(kf == FC - 1))
                            if g == 0:
                                nc.scalar.activation(
                                    y2_sb[:, 0:4, :ttl_c],
                                    ps_h4[:, :, :ttl_c],
                                    mybir.ActivationFunctionType.Copy)
                            else:
                                nc.vector.tensor_copy(
                                    y2_sb[:, 4:8, :ttl_c],
                                    ps_h4[:, :, :ttl_c])
                        nc.sync.dma_start(
                            yg2_r[:, :, po:po + ttl_c],
                            y2_sb[:, :, :ttl_c])
                        continue
                    tt0 = j * 128
                    ttl = min(128, tl - tt0)
                    wj = w_sb[:ttl, jg + j:jg + j + 1]
                    rows = slice(t0 + tt0, t0 + tt0 + ttl)
                    y_sb = y_p.tile([128, H], BF16, tag="y", name=f"y_sb_{jg+j}")
                    if final and j == nt - 1:
                        # final tile: independent PSUM groups per drain piece
                        # so each piece's evac+DMA chain starts as soon as its
                        # own matmuls finish (parallel engines/queues)
                        ps_a = ps_pool.tile([128, 512], F32, tag="psy",
                                            bufs=3, name="ps_fa")
                        for kf in range(FC):
                            nc.tensor.matmul(
                                ps_a[:ttl],
                                lhsT=ht_sb[:, kf, tt0:tt0 + ttl],
                                rhs=w2_sb[:, kf, 0:512],
                                start=(kf == 0), stop=(kf == FC - 1))
                        nc.vector.tensor_scalar(
                            y_sb[:ttl, 0:512], ps_a[:ttl], wj, None,
                            op0=mybir.AluOpType.mult)
                        nc.sync.dma_start(
                            yg[rows, 0:512], y_sb[:ttl, 0:512])
                        ps_b0 = ps_pool.tile([128, 256], F32, tag="psr",
                                             bufs=2, name="ps_fb0")
                        for kf in range(FC):
                            nc.tensor.matmul(
                                ps_b0[:ttl],
                                lhsT=ht_sb[:, kf, tt0:tt0 + ttl],
                                rhs=w2_sb[:, kf, 512:768],
                                start=(kf == 0), stop=(kf == FC - 1))
                        nc.scalar.activation(
                            y_sb[:ttl, 512:768], ps_b0[:ttl, :],
                            mybir.ActivationFunctionType.Copy, scale=wj)
                        nc.scalar.dma_start(
                            yg[rows, 512:768], y_sb[:ttl, 512:768])
                        ps_b1 = ps_pool.tile([128, 256], F32, tag="psr",
                                             bufs=2, name="ps_fb1")
                        for kf in range(FC):
                            nc.tensor.matmul(
                                ps_b1[:ttl],
                                lhsT=ht_sb[:, kf, tt0:tt0 + ttl],
                                rhs=w2_sb[:, kf, 768:1024],
                                start=(kf == 0), stop=(kf == FC - 1))
                        ybq = y_p.tile([128, 256], BF16, tag="ybq",
                                       bufs=1, name="ybq_last")
                        nc.vector.tensor_scalar(
                            ybq[:ttl, :], ps_b1[:ttl, :],
                            wj, None, op0=mybir.AluOpType.mult)
                        nc.sync.dma_start(
                            yg[rows, 768:1024], ybq[:ttl, :])
                        continue
                    for hh in range(2):
                        ps_y = ps_pool.tile([128, 512], F32, tag="psy",
                                            bufs=3, name=f"ps_y_{jg+j}_{hh}")
                        for kf in range(FC):
                            nc.tensor.matmul(
                                ps_y[:ttl],
                                lhsT=ht_sb[:, kf, tt0:tt0 + ttl],
                                rhs=w2_sb[:, kf, hh * 512:(hh + 1) * 512],
                                start=(kf == 0), stop=(kf == FC - 1))
                        nc.vector.tensor_scalar(
                            y_sb[:ttl, hh * 512:(hh + 1) * 512],
                            ps_y[:ttl], wj, None,
                            op0=mybir.AluOpType.mult)
                    nc.sync.dma_start(
                        yg[rows, :], y_sb[:ttl, :])

            jglob = 0
            pending = None
            for s in range(S):
                cap = caps[s]
                off = sum(caps[:s])
                chunks = chunk_lists[s]

                # k=0 weights split into g/u halves so the first matmul's DMA
                # dependency is small; k>=1 combined to halve the issue count
                w13g0 = w13_p.tile([128, F], BF16, tag="w13g0",
                                   name=f"w13g0_{s}")
                w13u0 = w13_p.tile([128, F], BF16, tag="w13u0",
                                   name=f"w13u0_{s}")
                nc.sync.dma_start(w13g0[:], w13[s, 0:128, 0:F])
                nc.sync.dma_start(w13u0[:], w13[s, 0:128, F:2 * F])
                w13k = [None] + [w13_p.tile([128, 2 * F], BF16,
                                            tag=f"w13_{k}",
                                            name=f"w13_sb_{s}_{k}")
                                 for k in range(1, KC)]
                for k in range(1, KC):
                    nc.sync.dma_start(
                        w13k[k][:], w13[s, k * 128:(k + 1) * 128, :])

                def gv(k, fi):
                    if k == 0:
                        return w13g0[:, fi * 128:(fi + 1) * 128]
                    return w13k[k][:, fi * 128:(fi + 1) * 128]

                def uv(k, fi):
                    if k == 0:
                        return w13u0[:, fi * 128:(fi + 1) * 128]
                    return w13k[k][:, F + fi * 128:F + (fi + 1) * 128]

                tch0 = 0
                for ci, tl in enumerate(chunks):
                    t0 = off + tch0
                    tch0 += tl
                    nt = math.ceil(tl / 128)
                    ramp = (s == 0 and ci == 0)

                    xg_big = xg_p.tile([128, KC, 512], BF16, tag="xg",
                                       name=f"xg_sb_{s}_{ci}")
                    xg_sb = [xg_big[:, k, :] for k in range(KC)]
                    if ramp:
                        # per-k DMAs so the PE can consume k-chunks as they
                        # stream in during the cold start
                        for k in range(KC):
                            nc.scalar.dma_start(
                                xg_big[:, k, :tl], xgt_r[:, k, t0:t0 + tl])
                    else:
                        nc.sync.dma_start(
                            xg_big[:, :, :tl], xgt_r[:, :, t0:t0 + tl])
                    if ramp:
                        # routing consts + slot-0 w2 AFTER the ramp-critical
                        # xg tiles (a big early w2 transfer would stall the
                        # first matmuls behind it on the shared DMA engines)
                        nc.scalar.dma_start(
                            gt_sb[:],
                            gtp.rearrange("(ko p) e -> p ko e", p=128))
                        nc.scalar.dma_start(bias_sb[:], biasp[:])
                        if conv:
                            nc.scalar.dma_start(ident_sb[:], identf[:])
                    if ci == 0:
                        w2_sb = w2_p.tile([128, FC, H], BF16, tag="w2",
                                          name=f"w2_sb_{s}")
                        nc.scalar.dma_start(
                            w2_sb[:],
                            w2t[s].rearrange("(ko p) h -> p ko h", p=128))

                    ht_sb = ht_p.tile([128, FC, 512], BF16, tag="ht")

                    if not ramp:
                        # routing first: its ACT+DVE chain then completes
                        # during stage 1, well before stage 2 consumes w_sb
                        routing(xg_sb, tl, nt, s, ci, jglob)

                    # ---- stage 1: hT[f,t] = silu(x@w1.T).T * (x@w3.T).T ----
                    if ramp:
                        # k OUTER across all fi: the PE consumes each
                        # weight/activation k-chunk as it streams in.
                        # 8 live PSUM tiles across the three tags.
                        tags = ["ps1", "ps1", "ps1", "psy",
                                "psy", "psy", "psr", "psr"]
                        ps8 = [ps_pool.tile([128, 512], F32, tag=tags[i],
                                            bufs=(2 if tags[i] == "psr"
                                                  else 3),
                                            name=f"ps_ramp_{i}")
                               for i in range(8)]
                        ps_gs = ps8[0::2]
                        ps_us = ps8[1::2]
                        for k in range(KC):
                            for fi in range(FC):
                                nc.tensor.matmul(
                                    ps_gs[fi][:, :tl], lhsT=gv(k, fi),
                                    rhs=xg_sb[k][:, :tl],
                                    start=(k == 0), stop=(k == KC - 1))
                                nc.tensor.matmul(
                                    ps_us[fi][:, :tl], lhsT=uv(k, fi),
                                    rhs=xg_sb[k][:, :tl],
                                    start=(k == 0), stop=(k == KC - 1))
                        for fi in range(FC):
                            evac_stage1(ps_gs[fi], ps_us[fi], ht_sb, fi, tl)
                        # ramp routing last (needs every xg k-chunk anyway)
                        routing(xg_sb, tl, nt, s, ci, jglob)
                    else:
                        # fi sequential, k inner: only 2 PSUM tiles live
                        for fi in range(FC):
                            ps_g = ps_pool.tile([128, 512], F32, tag="ps1",
                                                bufs=3,
                                                name=f"ps_g_{s}_{ci}_{fi}")
                            ps_u = ps_pool.tile([128, 512], F32, tag="ps1",
                                                bufs=3,
                                                name=f"ps_u_{s}_{ci}_{fi}")
                            for k in range(KC):
                                nc.tensor.matmul(
                                    ps_g[:, :tl], lhsT=gv(k, fi),
                                    rhs=xg_sb[k][:, :tl],
                                    start=(k == 0), stop=(k == KC - 1))
                                nc.tensor.matmul(
                                    ps_u[:, :tl], lhsT=uv(k, fi),
                                    rhs=xg_sb[k][:, :tl],
                                    start=(k == 0), stop=(k == KC - 1))
                            evac_stage1(ps_g, ps_u, ht_sb, fi, tl)

                    # ---- stage 2 of the PREVIOUS chunk (pipelined) ----
                    if pending is not None:
                        emit_stage2(pending, final=False)
                    pending = {"tl": tl, "nt": nt, "t0": t0, "ht_sb": ht_sb,
                               "w2_sb": w2_sb, "jglob": jglob}
                    if ci == len(chunks) - 1 and s in conv_map:
                        # broadcast the tail tile's combine weights along the
                        # free dim (PE transpose + rank-1 matmul), pre-scale
                        # ht; its stage 2 then runs h-on-partitions with
                        # matmul cost proportional to the real token count
                        ttl_c, po = conv_map[s]
                        tt0p = (nt - 1) * 128
                        jcol = jglob + nt - 1
                        wrow_ps = ps_pool.tile([128, 512], F32, tag="psr",
                                               bufs=2, name=f"wrow_ps_{s}")
                        nc.tensor.transpose(
                            wrow_ps[0:1, 0:128], w_sb[:, jcol:jcol + 1],
                            ident_sb[:])
                        wrow_sb = work_p.tile([1, 128], F32, tag="wrow",
                                              name=f"wrow_sb_{s}")
                        nc.vector.tensor_copy(
                            wrow_sb[0:1, :], wrow_ps[0:1, 0:128])
                        wb_ps = ps_pool.tile([128, 512], F32, tag="psr",
                                             bufs=2, name=f"wb_ps_{s}")
                        nc.tensor.matmul(
                            wb_ps[:, :ttl_c], lhsT=ones_sb[0:1, :],
                            rhs=wrow_sb[0:1, :ttl_c], start=True, stop=True)
                        wbs = work_p.tile([128, 128], F32, tag="wbs",
                                          name=f"wbs_{s}")
                        nc.vector.tensor_copy(
                            wbs[:, :ttl_c], wb_ps[:, :ttl_c])
                        htw = ht_p.tile([128, FC, 128], BF16, tag="htw",
                                        name=f"htw_{s}")
                        nc.vector.tensor_tensor(
                            htw[:, :, :ttl_c],
                            ht_sb[:, :, tt0p:tt0p + ttl_c],
                            wbs[:, None, :ttl_c].to_broadcast(
                                [128, FC, ttl_c]),
                            mybir.AluOpType.mult)
                        pending["conv"] = (ttl_c, po, htw)
                    jglob += nt

            emit_stage2(pending, final=True)

    nc.compile()
    return nc


def _moe_nc(caps):
    key = ("moe", caps)
    if key not in _nc_cache:
        _nc_cache[key] = _build_moe(caps)
    return _nc_cache[key]


def kernel(hidden_states, gate_w, bias, w1, w3, w2):
    x = np.ascontiguousarray(np.asarray(hidden_states, dtype=np.float32))
    gate_w = np.asarray(gate_w, dtype=np.float32)
    bias = np.asarray(bias, dtype=np.float32)
    w1 = np.asarray(w1, dtype=np.float32)
    w3 = np.asarray(w3, dtype=np.float32)
    w2 = np.asarray(w2, dtype=np.float32)

    # ---- Host dispatch: fp32 routing decides token->expert placement ----
    logits = x @ gate_w.T                                # [T, E]
    scores = 1.0 / (1.0 + np.exp(-logits))
    biased = scores + bias[None, :]
    topi = np.argpartition(-biased, TOPK - 1, axis=1)[:, :TOPK]  # [T, K] sets
    sel = np.zeros((T, E), dtype=bool)
    sel[np.arange(T)[:, None], topi] = True
    idx_per_e = [np.nonzero(sel[:, e])[0] for e in range(E)]
    counts = np.array([len(ix) for ix in idx_per_e])
    caps, placement = _plan_slots(counts)
    S = len(caps)
    offs = [sum(caps[:si]) for si in range(S)]
    global LAST_CAPS
    LAST_CAPS = caps
    CT = sum(caps)

    xT = np.ascontiguousarray(x.T)                       # [H, T]
    xT16 = xT.astype(ml_dtypes.bfloat16)
    gT16 = np.ascontiguousarray(gate_w.T).astype(ml_dtypes.bfloat16)

    in_maps = []
    for c in range(NCORES):
        slot_experts = [p[0] for p in placement[c]]
        idx_pad = np.zeros(CT, dtype=np.int64)
        for si, (e, st, ln) in enumerate(placement[c]):
            if ln:
                idx_pad[offs[si]:offs[si] + ln] = idx_per_e[e][st:st + ln]
        xgt = np.ascontiguousarray(xT16[:, idx_pad])     # [H, CT] bf16
        w13t = np.stack([
            np.ascontiguousarray(
                np.concatenate([w1[e].T, w3[e].T], axis=1))
            for e in slot_experts]).astype(ml_dtypes.bfloat16)  # [S, H, 2F]
        w2t = np.stack(
            [np.ascontiguousarray(w2[e].T) for e in slot_experts]
        ).astype(ml_dtypes.bfloat16)
        perm = slot_experts + [e for e in range(E) if e not in slot_experts]
        gtp = np.ascontiguousarray(gT16[:, perm])        # [H, E] bf16
        biasp = np.ascontiguousarray(
            np.broadcast_to(np.asarray(bias)[perm][None, :],
                            (128, E))).astype(np.float32)
        im = {"w13t": w13t, "w2t": w2t, "xgt": xgt, "gtp": gtp,
              "biasp": biasp}
        if _conv_tiles(caps):
            im["identf"] = np.eye(128, dtype=np.float32)
        in_maps.append(im)

    # ---- Single SPMD launch: routing weights + expert FFN ----
    ncB = _moe_nc(caps)
    res = run_bass_kernel_spmd(ncB, in_maps, core_ids=list(range(NCORES)))

    # ---- Host combine: scatter-add (tail-tile rows come from yg2) ----
    conv_map = {}
    po = 0
    for s_c, ttl_c in _conv_tiles(caps):
        conv_map[s_c] = (ttl_c, po)
        po += ttl_c
    out = np.zeros((T, H), dtype=np.float32)
    for c in range(NCORES):
        yg_c = res.results[c]["yg"]
        yg2_c = res.results[c].get("yg2")
        for si, (e, st, ln) in enumerate(placement[c]):
            if not ln:
                continue
            ix = idx_per_e[e][st:st + ln]
            t0p = (math.ceil(caps[si] / 128) - 1) * 128
            if si in conv_map and ln > t0p:
                ttl_c, po = conv_map[si]
                seg = np.concatenate([
                    yg_c[offs[si]:offs[si] + t0p].astype(np.float32),
                    yg2_c[:, po:po + (ln - t0p)].T.astype(np.float32)],
                    axis=0)
            else:
                seg = yg_c[offs[si]:offs[si] + ln].astype(np.float32)
            out[ix] += seg
    return out
